# revision 3
# baseline (speedup 1.0000x reference)
"""Exact entmax-1.5 loss kernel for Trainium2 (8 NeuronCores, data-parallel over rows).

v2 pipeline per 128-row block (fp16 payloads/group-maxes):
  A. Stream X in 20 f32 1600-col chunks (triple-buffered, DMA-bound pace);
     scalar converts to resident fp16 xh; vector builds group-of-4 maxes via a
     2-op max tree (strided pair-max into de-interleaved scratch, then a
     unit-stride 2x-mode max) and per-chunk group-of-64 maxes (k=16 reduce).
     At chunk 14 a 5-iteration partial Newton on g64[:,0:375] warm-starts th.
  B. 2 full Newton iterations on g64[:,0:500] -> th (lower bound on theta*).
  C. mask = relu(g4 - (th-EPS)) in place; 16 accumulating f16 matmuls with
     residue-selection weights -> per-cluster group activity in wrapped
     [128,500] PSUM layout.
  D. Compaction: iota*mask, scan -> ranks, local_scatter -> 60 gids per
     partition, sentinel fixup.
  E. Two ap_gathers (640+320 idxs) pull 4-fp16 payloads of the cluster-union
     candidate groups from xh -> cand [128,3840] fp16.
  F. Taylor-fused stats: ONE pass over cand computing S1,S2 (scalar activation
     accumulators) and S3 (vector mult+reduce) at th, then a Newton-corrected
     closed form: delta=(S2-4)/(2*S1), S3c=S3-3*delta*(S2-S1*delta),
     loss = 4/3 + S3c/12 + (th+delta) - x_t.  (d loss/d th = 1 - S2/4 -> 0 at
     theta*, so the single correction suffices to ~1e-6.)

Host wrapper shards rows 256-per-core across 8 cores, no collectives.
"""
import numpy as np
from contextlib import ExitStack

N, V = 2048, 32000
N_CORES = 8
ROWS = N // N_CORES          # 256 rows per core
CHUNK = 1600
NCHUNK = V // CHUNK          # 20
NG4 = V // 4                 # 8000 groups of 4
G4C = CHUNK // 4             # 400 g4 cols per chunk
G64C = G4C // 16             # 25 g64 cols per chunk
WF = 500                     # wrapped cols: group = 16*F + (p % 16)
PART_ITERS = 5               # partial-newton iters on first 375 g64 cols
FULL_ITERS = 2
EPS_LB = 2e-2                # X-units safety margin (covers f16 bound noise)
S4 = 60                      # per-partition capacity of compacted group ids
KU = 16 * S4                 # 960 union groups per 16-partition cluster
CW = 4 * KU                  # 3840 compact width (fp16)
GPAD = 4
NGP = NG4 + GPAD             # 8004 groups incl. sentinel pad
DW = 4 * NGP                 # 32016 fp16 X width
SENT = NG4 + 2               # sentinel group id for scatter pads
EA = 40                      # slot split: gather A = slots [0:40] -> cand[0:2560]
FSL = 960                    # F-phase slice width
NEG = -60000.0               # fp16-safe -inf substitute

_nc_cache = {}


def _build_nc():
    import concourse.bass as bass
    import concourse.bacc as bacc
    import concourse.tile as tile
    from concourse import mybir

    f32 = mybir.dt.float32
    f16 = mybir.dt.float16
    i16 = mybir.dt.int16
    Alu = mybir.AluOpType
    Act = mybir.ActivationFunctionType
    Ax = mybir.AxisListType

    nc = bacc.Bacc("TRN2", target_bir_lowering=False, debug=False)
    x = nc.dram_tensor("x", [ROWS, V], f32, kind="ExternalInput").ap()
    oh = nc.dram_tensor("oh", [ROWS, 64], f32, kind="ExternalInput").ap()
    tbl = nc.dram_tensor("tbl", [128, 16], i16, kind="ExternalInput").ap()
    iotd = nc.dram_tensor("iot", [128, WF], i16, kind="ExternalInput").ap()
    wseld = nc.dram_tensor("wsel", [128, 16 * 128], f16, kind="ExternalInput").ap()
    out = nc.dram_tensor("loss", [ROWS], f32, kind="ExternalOutput").ap()

    with tile.TileContext(nc) as tc, ExitStack() as ctx:
        const = ctx.enter_context(tc.tile_pool(name="const", bufs=1))
        big = ctx.enter_context(tc.tile_pool(name="big", bufs=1))
        psum = ctx.enter_context(tc.tile_pool(name="psum", bufs=2, space="PSUM"))

        iot = const.tile([128, WF], i16, tag="iot")
        wt = const.tile([128, 16 * 128], f16, tag="wsel")
        xtg = const.tile([128, 2], f32, tag="xtg")
        tbl_t = const.tile([128, 16], i16, tag="tbl")

        nc.sync.dma_start(tbl_t[:], tbl[:, :])

        # ---- x[target] DMA gathers for both blocks, up front (gpsimd mlp lib).
        gtiles = []
        for b in range(2):
            gA = big.tile([128, 64], f32, tag="gA", bufs=2)
            gB = big.tile([128, 64], f32, tag="gB", bufs=2)
            for half, gdst in ((0, gA), (1, gB)):
                c = 2 * b + half
                src = x[64 * c : 64 * (c + 1), :].rearrange(
                    "r (bk e) -> (r bk) e", e=64
                )
                nc.gpsimd.dma_gather(
                    gdst.rearrange("p (one e) -> p one e", one=1),
                    src,
                    tbl_t[:, 4 * c : 4 * (c + 1)],
                    num_idxs=64,
                    num_idxs_reg=64,
                    elem_size=64,
                )
            nc.gpsimd.dma_start(gA[64:128, :], gB[0:64, :])
            nc.gpsimd.dma_start(gB[:], oh[bass.ts(b, 128), :])
            gtiles.append((gA, gB))

        # dummy 16-idx ap_gather: pre-loads the Q7 ap_gather library
        sc0_w = big.tile([128, 80], f32, tag="warm", bufs=1)
        wix = sc0_w[:, 79:80].bitcast(i16)[:, 0:1]
        nc.vector.memset(wix, 0)
        nc.gpsimd.ap_gather(
            sc0_w[:, 0:64].rearrange("p (a d) -> p a d", d=4),
            gtiles[1][1][:, 0:16].rearrange("p (a d) -> p a d", d=4),
            wix,
            channels=128,
            num_elems=4,
            d=4,
            num_idxs=16,
        )

        # per-block tiles (tag rotation gives block-alternating buffers)
        tiles = []
        for b in range(2):
            t = {}
            t["xh"] = big.tile([128, DW], f16, name="xh", tag="xh", bufs=2)
            t["g4"] = big.tile([128, NG4], f16, name="g4", tag="g4", bufs=2)
            t["g64"] = big.tile([128, WF], f16, name="g64", tag="g64", bufs=2)
            t["cand"] = big.tile([128, CW], f16, name="cand", tag="cand", bufs=2)
            t["sc"] = big.tile([128, 128], f32, name="sc", tag="sc", bufs=2)
            tiles.append(t)
        wk = big.tile([128, FSL], f16, name="wk", tag="wk", bufs=1)
        scr = big.tile([128, 768], f32, name="scr", tag="scr", bufs=1)

        def newton_iters(t, width, iters):
            sc = t["sc"]
            th = sc[:, 1:2]
            nuB = sc[:, 2:3]
            S1a = sc[:, 3:4]
            S2a = sc[:, 4:5]
            r1 = sc[:, 7:8]
            dd = sc[:, 8:9]
            uB = scr[:, 250:500].bitcast(f16)
            g = t["g64"]
            for _ in range(iters):
                nc.vector.tensor_scalar(out=nuB, in0=th, scalar1=-1.0, scalar2=None, op0=Alu.mult)
                nc.scalar.activation(
                    uB[:, 0:width], g[:, 0:width], Act.Relu, bias=nuB, scale=1.0,
                    accum_out=S1a,
                )
                nc.scalar.activation(uB[:, 0:width], uB[:, 0:width], Act.Square, accum_out=S2a)
                nc.vector.reciprocal(r1, S1a)
                nc.vector.tensor_scalar(
                    out=dd, in0=S2a, scalar1=-4.0, scalar2=0.5, op0=Alu.add, op1=Alu.mult
                )
                nc.vector.scalar_tensor_tensor(
                    out=th, in0=dd, scalar=r1, in1=th, op0=Alu.mult, op1=Alu.add
                )

        def phase_A(b):
            t = tiles[b]
            xh, g4, g64, sc = t["xh"], t["g4"], t["g64"], t["sc"]
            m_s = sc[:, 0:1]
            th = sc[:, 1:2]
            nc.vector.memset(xh[:, V:DW], NEG)
            xb = x[bass.ts(b, 128), :]
            for c in range(NCHUNK):
                xc = big.tile([128, CHUNK], f32, tag="xc", bufs=3)
                nc.sync.dma_start(xc[:], xb[:, bass.ts(c, CHUNK)])
                nc.scalar.activation(
                    xh[:, bass.ts(c, CHUNK)], xc[:], Act.Copy, bias=0.0, scale=1.0
                )
                # 2-op group-of-4 max tree:
                #   t2[400*a + g] = max(x[4g+2a], x[4g+2a+1])   (strided pair max)
                #   g4[g] = max(t2[g], t2[400+g])               (unit-stride, 2x mode)
                t2 = big.tile([128, 2 * G4C], f16, tag="t2", bufs=2)
                xv = xh[:, bass.ts(c, CHUNK)].rearrange("p (g a two) -> p g a two", a=2, two=2)
                nc.vector.tensor_tensor(
                    out=t2[:].rearrange("p (a g) -> p g a", a=2),
                    in0=xv[:, :, :, 0],
                    in1=xv[:, :, :, 1],
                    op=Alu.max,
                )
                nc.vector.tensor_tensor(
                    out=g4[:, bass.ts(c, G4C)],
                    in0=t2[:, 0:G4C],
                    in1=t2[:, G4C : 2 * G4C],
                    op=Alu.max,
                )
                nc.vector.tensor_reduce(
                    g64[:, bass.ts(c, G64C)],
                    g4[:, bass.ts(c, G4C)].rearrange("p (g k) -> p g k", k=16),
                    axis=Ax.X,
                    op=Alu.max,
                )
                if c == 14:
                    # partial-newton warm start on g64 cols 0:375 (chunks 0-14)
                    nc.vector.tensor_reduce(m_s, g64[:, 0:375], axis=Ax.X, op=Alu.max)
                    nc.vector.tensor_scalar(
                        out=th, in0=m_s, scalar1=-2.0, scalar2=None, op0=Alu.add
                    )
                    newton_iters(t, 375, PART_ITERS)

        def phase_BCDE(b):
            t = tiles[b]
            g4, sc = t["g4"], t["sc"]

            th = sc[:, 1:2]
            nu = sc[:, 2:3]
            th_m = sc[:, 9:10]
            vcomp = sc[:, 16:48].bitcast(i16)      # [128, 64] i16
            bneg = sc[:, 48:112]                    # [128, 64] f32

            maskv = scr[:, 0:250].bitcast(f16)      # [128, 500] f16
            cum = scr[:, 250:500].bitcast(f16)      # [128, 500] f16
            v16 = scr[:, 500:756].bitcast(i16)      # [128, 512] i16
            rank = scr[:, 0:256].bitcast(i16)       # [128, 512] i16, reuses maskv

            # ---- B: finish the newton on the full 500 cols ----
            newton_iters(t, WF, FULL_ITERS)
            nc.vector.tensor_scalar(out=th_m, in0=th, scalar1=-EPS_LB, scalar2=None, op0=Alu.add)
            nc.vector.tensor_scalar(out=nu, in0=th, scalar1=-1.0, scalar2=None, op0=Alu.mult)

            # ---- C: candidate mask (in place on g4) + residue matmuls ----
            nc.vector.tensor_scalar(
                out=g4[:], in0=g4[:], scalar1=th_m, scalar2=0.0,
                op0=Alu.subtract, op1=Alu.max,
            )
            pc = psum.tile([128, WF], f32, tag="pc")
            g4v = g4[:].rearrange("p (f wr) -> p f wr", wr=16)
            for w in range(16):
                nc.tensor.matmul(
                    pc[:],
                    wt[:, bass.ts(w, 128)],
                    g4v[:, :, w],
                    start=(w == 0),
                    stop=(w == 15),
                )

            # ---- D: compaction ----
            nc.vector.tensor_scalar(out=maskv, in0=pc[:], scalar1=0.0, scalar2=None, op0=Alu.is_gt)
            nc.vector.scalar_tensor_tensor(
                out=v16[:, 0:WF], in0=maskv, scalar=1.0, in1=iot[:],
                op0=Alu.mult, op1=Alu.mult,
            )
            nc.vector.tensor_tensor_scan(
                out=cum, data0=maskv, data1=maskv, initial=0.0,
                op0=Alu.add, op1=Alu.bypass,
            )
            nc.vector.tensor_tensor(out=cum, in0=cum, in1=maskv, op=Alu.mult)
            nc.vector.scalar_tensor_tensor(
                out=cum, in0=cum, scalar=float(S4) + 0.5, in1=cum,
                op0=Alu.is_le, op1=Alu.mult,
            )
            nc.vector.tensor_scalar(out=rank[:, 0:WF], in0=cum, scalar1=-1.0, scalar2=None, op0=Alu.add)
            nc.gpsimd.local_scatter(
                vcomp[:, 0:S4],
                v16[:, 0:WF],
                rank[:, 0:WF],
                channels=128,
                num_elems=S4,
                num_idxs=WF,
            )
            # group idx = (gid+1) - 1; scatter pads (0) map to sentinel group
            nc.vector.tensor_scalar(
                out=bneg[:, 0:S4], in0=vcomp[:, 0:S4], scalar1=0.5, scalar2=float(SENT) + 1.0,
                op0=Alu.is_lt, op1=Alu.mult,
            )
            nc.vector.scalar_tensor_tensor(
                out=bneg[:, 0:S4], in0=vcomp[:, 0:S4], scalar=1.0, in1=bneg[:, 0:S4],
                op0=Alu.mult, op1=Alu.add,
            )
            gix = v16[:, 0:S4]
            nc.vector.tensor_scalar(out=gix, in0=bneg[:, 0:S4], scalar1=-1.0, scalar2=None, op0=Alu.add)

            # ---- E: gather candidate payloads from xh (two slot ranges) ----
            cand, xh = t["cand"], t["xh"]
            for lo_s, hi_s in ((0, EA), (EA, S4)):
                nc.gpsimd.ap_gather(
                    cand[:, 64 * lo_s : 64 * hi_s].rearrange("p (a d) -> p a d", d=4),
                    xh[:].rearrange("p (a d) -> p a d", d=4),
                    gix[:, lo_s:hi_s],
                    channels=128,
                    num_elems=DW // 4,
                    d=4,
                    num_idxs=16 * (hi_s - lo_s),
                )

        def phase_F(b):
            t = tiles[b]
            cand, sc = t["cand"], t["sc"]
            gA, gB = gtiles[b]
            th = sc[:, 1:2]
            nu = sc[:, 2:3]
            r1 = sc[:, 7:8]
            one_t = sc[:, 0:1]
            S1s = [sc[:, 20 + i : 21 + i] for i in range(4)]
            S2s = [sc[:, 24 + i : 25 + i] for i in range(4)]
            S3s = [sc[:, 28 + i : 29 + i] for i in range(4)]
            S1 = sc[:, 3:4]
            S2 = sc[:, 4:5]
            S3 = sc[:, 5:6]
            dlt = sc[:, 8:9]
            e1 = sc[:, 10:11]
            tb_ = sc[:, 14:15]
            lo = sc[:, 15:16]

            # single fused stats pass at theta0 = th: S1,S2 via scalar
            # activation accumulators; S3 via vector mult+reduce
            for i in range(4):
                sl = slice(FSL * i, FSL * (i + 1))
                nc.scalar.activation(
                    wk[:], cand[:, sl], Act.Relu, bias=nu, scale=1.0, accum_out=S1s[i]
                )
                nc.scalar.activation(cand[:, sl], wk[:], Act.Square, accum_out=S2s[i])
                nc.vector.tensor_tensor(out=cand[:, sl], in0=wk[:], in1=cand[:, sl], op=Alu.mult)
                nc.vector.tensor_reduce(S3s[i], cand[:, sl], axis=Ax.X, op=Alu.add)
            for dst, parts in ((S1, S1s), (S2, S2s), (S3, S3s)):
                nc.vector.tensor_tensor(out=dst, in0=parts[0], in1=parts[1], op=Alu.add)
                nc.vector.tensor_tensor(out=parts[2], in0=parts[2], in1=parts[3], op=Alu.add)
                nc.vector.tensor_tensor(out=dst, in0=dst, in1=parts[2], op=Alu.add)

            # Newton-corrected closed form
            nc.vector.reciprocal(r1, S1)
            nc.vector.tensor_scalar(
                out=dlt, in0=S2, scalar1=-4.0, scalar2=0.5, op0=Alu.add, op1=Alu.mult
            )
            nc.vector.tensor_tensor(out=dlt, in0=dlt, in1=r1, op=Alu.mult)

            # x[target] one-hot dot; one_t (==1.0) anchored on r1 so the static
            # scheduler cannot hoist these to the head of the queue
            nc.vector.tensor_scalar(
                out=one_t, in0=r1, scalar1=0.0, scalar2=1.0, op0=Alu.mult, op1=Alu.add
            )
            nc.vector.scalar_tensor_tensor(
                out=gB[:], in0=gA[:], scalar=one_t, in1=gB[:], op0=Alu.mult, op1=Alu.mult
            )
            nc.vector.tensor_reduce(xtg[:, b : b + 1], gB[:], axis=Ax.X, op=Alu.add)

            # S3c = S3 - 3*dlt*(S2 - S1*dlt);  loss = 4/3 + S3c/12 + th + dlt - x_t
            nc.vector.tensor_tensor(out=e1, in0=S1, in1=dlt, op=Alu.mult)
            nc.vector.tensor_tensor(out=e1, in0=S2, in1=e1, op=Alu.subtract)
            nc.vector.tensor_tensor(out=e1, in0=e1, in1=dlt, op=Alu.mult)
            nc.vector.scalar_tensor_tensor(
                out=S3, in0=e1, scalar=-3.0, in1=S3, op0=Alu.mult, op1=Alu.add
            )
            nc.vector.tensor_tensor(out=tb_, in0=th, in1=dlt, op=Alu.add)
            nc.vector.scalar_tensor_tensor(
                out=tb_, in0=S3, scalar=1.0 / 12.0, in1=tb_, op0=Alu.mult, op1=Alu.add
            )
            nc.vector.scalar_tensor_tensor(
                out=lo, in0=tb_, scalar=4.0 / 3.0, in1=xtg[:, b : b + 1],
                op0=Alu.add, op1=Alu.subtract,
            )
            nc.sync.dma_start(out[bass.ts(b, 128)], lo)

        phase_A(0)
        nc.sync.dma_start(iot[:], iotd)
        nc.sync.dma_start(wt[:], wseld)
        phase_BCDE(0)
        phase_A(1)
        phase_F(0)
        phase_BCDE(1)
        phase_F(1)

    nc.compile()
    return nc


def get_nc():
    if "nc" not in _nc_cache:
        _nc_cache["nc"] = _build_nc()
    return _nc_cache["nc"]


def make_in_maps(X, target):
    import ml_dtypes

    X = np.ascontiguousarray(np.asarray(X, dtype=np.float32))
    target = np.asarray(target).astype(np.int64)

    # wrapped gid+1 iota: iot[p, f] = 16*f + (p % 16) + 1
    pp, ff = np.meshgrid(np.arange(128), np.arange(WF), indexing="ij")
    iot = (16 * ff + (pp % 16) + 1).astype(np.int16)
    # residue-selection matrices: wsel[p, w, n] = 1 if n == 16*(p//16) + w
    wsel = np.zeros((128, 16, 128), np.float32)
    for w in range(16):
        for p in range(128):
            wsel[p, w, 16 * (p // 16) + w] = 1.0
    wsel = wsel.reshape(128, 16 * 128).astype(np.float16)

    in_maps = []
    for k in range(N_CORES):
        Xk = X[k * ROWS : (k + 1) * ROWS]
        tk = target[k * ROWS : (k + 1) * ROWS]
        ohk = np.zeros((ROWS, 64), np.float32)
        ohk[np.arange(ROWS), (tk % 64).astype(np.int64)] = 1.0
        tblk = np.zeros((128, 16), np.int16)
        for c in range(4):
            rows = np.arange(64)
            vals = (rows * (V // 64) + (tk[64 * c + rows] // 64)).astype(np.int16)
            w = np.zeros((16, 4), np.int16)
            w[rows % 16, rows // 16] = vals
            tblk[:, 4 * c : 4 * (c + 1)] = np.tile(w, (8, 1))
        in_maps.append({"x": Xk, "oh": ohk, "tbl": tblk, "iot": iot, "wsel": wsel})
    return in_maps


def kernel(X, target):
    from concourse.bass_utils import run_bass_kernel_spmd

    nc = get_nc()
    in_maps = make_in_maps(X, target)
    res = run_bass_kernel_spmd(nc, in_maps, core_ids=list(range(N_CORES)))
    loss = np.concatenate([r["loss"] for r in res.results]).astype(np.float32)
    return loss


# revision 6
# speedup vs baseline: 1.0124x; 1.0124x over previous
"""Exact entmax-1.5 loss kernel for Trainium2 (8 NeuronCores, data-parallel over rows).

v3: software-pipelined candidate extraction so the big gpsimd gathers run
under the DMA stream, plus a Taylor-fused single-pass stats phase.

Per 128-row block (fp16 payloads/group-maxes):
  A. Stream X in 20 f32 1600-col chunks (triple-buffered); scalar converts to
     resident fp16 xh; vector builds g4 (k=4 reduce) and per-chunk g64 (k=16);
     chunks {11,12,14} run their g4 on the gpsimd engine to offload vector.
     Progressive Newton lower bound on g64 prefixes: 5 iters @ width 250
     (chunk 10), 2 @ 350 (c13), 2 @ 425 (c16). At c16 the bound is tight
     enough to mask the A-half of the groups (f 0:250): relu-mask in place,
     16 residue matmuls -> wrapped PSUM activity, scan+local_scatter
     compaction -> 38 gids/partition, ap_gather (608 idxs) pulls the A-half
     candidate payloads DURING the remaining stream.
  B. After the stream: 2 full Newton iters -> mask-B bound; B-half (f 250:500)
     mask + matmuls + compaction (36 gids) + gather (576 idxs); a 3rd Newton
     iter runs concurrently (on the scalar/vector engines) with the B matmuls
     to refine theta for F.
  F. Taylor-fused stats: ONE pass over cand [128,4736] in 8 slices computing
     S1,S2 (scalar activation accumulators) and S3 (A-slices: vector
     mult+reduce; B-slices: gpsimd) at theta, then the Newton-corrected form
     delta=(S2-4)/(2*S1), S3c=S3-3*delta*(S2-S1*delta),
     loss = 4/3 + S3c/12 + (theta+delta) - x[target].

Host wrapper shards rows 256-per-core across 8 cores, no collectives.
"""
import numpy as np
from contextlib import ExitStack

N, V = 2048, 32000
N_CORES = 8
ROWS = N // N_CORES          # 256 rows per core
CHUNK = 1600
NCHUNK = V // CHUNK          # 20
NG4 = V // 4                 # 8000 groups of 4
G4C = CHUNK // 4             # 400 g4 cols per chunk
G64C = G4C // 16             # 25 g64 cols per chunk
WF = 500                     # wrapped cols: group = 16*F + (p % 16)
WH = 250                     # A/B split point in wrapped cols
EPS_LB = 2e-2                # X-units safety margin (covers f16 bound noise)
S4A = 38                     # per-partition compacted-gid capacity, A half
S4B = 36                     # and B half
CWA = 64 * S4A               # 2432 candidate cols from the A half
CWB = 64 * S4B               # 2304 from the B half
CW = CWA + CWB               # 4736
GPAD = 4
NGP = NG4 + GPAD             # 8004 groups incl. sentinel pad
DW = 4 * NGP                 # 32016 fp16 X width
SENT = NG4 + 2               # sentinel group id for scatter pads
GP_CHUNKS = ()               # gpsimd cannot run generic vector ops on trn2
NEG = -60000.0               # fp16-safe -inf substitute

_nc_cache = {}


def _build_nc():
    import concourse.bass as bass
    import concourse.bacc as bacc
    import concourse.tile as tile
    from concourse import mybir

    f32 = mybir.dt.float32
    f16 = mybir.dt.float16
    i16 = mybir.dt.int16
    Alu = mybir.AluOpType
    Act = mybir.ActivationFunctionType
    Ax = mybir.AxisListType

    nc = bacc.Bacc("TRN2", target_bir_lowering=False, debug=False)
    x = nc.dram_tensor("x", [ROWS, V], f32, kind="ExternalInput").ap()
    oh = nc.dram_tensor("oh", [ROWS, 64], f32, kind="ExternalInput").ap()
    tbl = nc.dram_tensor("tbl", [128, 16], i16, kind="ExternalInput").ap()
    iotd = nc.dram_tensor("iot", [128, WF], i16, kind="ExternalInput").ap()
    wseld = nc.dram_tensor("wsel", [128, 16 * 128], f16, kind="ExternalInput").ap()
    out = nc.dram_tensor("loss", [ROWS], f32, kind="ExternalOutput").ap()

    with tile.TileContext(nc) as tc, ExitStack() as ctx:
        const = ctx.enter_context(tc.tile_pool(name="const", bufs=1))
        big = ctx.enter_context(tc.tile_pool(name="big", bufs=1))
        psum = ctx.enter_context(tc.tile_pool(name="psum", bufs=2, space="PSUM"))

        iot = const.tile([128, WF], i16, tag="iot")
        wt = const.tile([128, 16 * 128], f16, tag="wsel")
        xtg = const.tile([128, 2], f32, tag="xtg")
        tbl_t = const.tile([128, 16], i16, tag="tbl")

        nc.sync.dma_start(tbl_t[:], tbl[:, :])

        # ---- x[target] DMA gathers for both blocks, up front (gpsimd mlp lib).
        gtiles = []
        for b in range(2):
            gA = big.tile([128, 64], f32, tag="gA", bufs=2)
            gB = big.tile([128, 64], f32, tag="gB", bufs=2)
            for half, gdst in ((0, gA), (1, gB)):
                c = 2 * b + half
                src = x[64 * c : 64 * (c + 1), :].rearrange(
                    "r (bk e) -> (r bk) e", e=64
                )
                nc.gpsimd.dma_gather(
                    gdst.rearrange("p (one e) -> p one e", one=1),
                    src,
                    tbl_t[:, 4 * c : 4 * (c + 1)],
                    num_idxs=64,
                    num_idxs_reg=64,
                    elem_size=64,
                )
            nc.gpsimd.dma_start(gA[64:128, :], gB[0:64, :])
            nc.gpsimd.dma_start(gB[:], oh[bass.ts(b, 128), :])
            gtiles.append((gA, gB))

        scr = big.tile([128, 640], f32, name="scr", tag="scr", bufs=1)

        # dummy 16-idx ap_gather: pre-loads the Q7 ap_gather library
        wix = scr[:, 99:100].bitcast(i16)[:, 0:1]
        nc.vector.memset(wix, 0)
        nc.gpsimd.ap_gather(
            scr[:, 0:64].rearrange("p (a d) -> p a d", d=4),
            gtiles[1][1][:, 0:16].rearrange("p (a d) -> p a d", d=4),
            wix,
            channels=128,
            num_elems=4,
            d=4,
            num_idxs=16,
        )

        # per-block tiles (tag rotation gives block-alternating buffers)
        tiles = []
        for b in range(2):
            t = {}
            t["xh"] = big.tile([128, DW], f16, name="xh", tag="xh", bufs=2)
            t["g4"] = big.tile([128, NG4], f16, name="g4", tag="g4", bufs=2)
            t["g64"] = big.tile([128, WF], f16, name="g64", tag="g64", bufs=2)
            t["cand"] = big.tile([128, CW], f16, name="cand", tag="cand", bufs=2)
            t["sc"] = big.tile([128, 96], f32, name="sc", tag="sc", bufs=2)
            tiles.append(t)
        wk = big.tile([128, 640], f16, name="wk", tag="wk", bufs=1)

        # scr layout (f32 cols): uB [0:250] (f16 500 newton scratch),
        # maskv [250:375] (f16 250), cum [375:500] (f16 250),
        # v16/gix [500:625] (i16 250), rank reuses maskv, bneg reuses cum.
        uB = scr[:, 0:250].bitcast(f16)
        maskv = scr[:, 250:375].bitcast(f16)
        cum = scr[:, 375:500].bitcast(f16)
        v16 = scr[:, 500:625].bitcast(i16)
        rank = scr[:, 250:375].bitcast(i16)
        bneg = scr[:, 375:500]

        def newton_iters(t, width, iters):
            sc = t["sc"]
            th = sc[:, 1:2]
            nuB = sc[:, 2:3]
            S1a = sc[:, 3:4]
            S2a = sc[:, 4:5]
            r1 = sc[:, 7:8]
            dd = sc[:, 8:9]
            g = t["g64"]
            for _ in range(iters):
                nc.vector.tensor_scalar(out=nuB, in0=th, scalar1=-1.0, scalar2=None, op0=Alu.mult)
                nc.scalar.activation(
                    uB[:, 0:width], g[:, 0:width], Act.Relu, bias=nuB, scale=1.0,
                    accum_out=S1a,
                )
                nc.scalar.activation(uB[:, 0:width], uB[:, 0:width], Act.Square, accum_out=S2a)
                nc.vector.reciprocal(r1, S1a)
                nc.vector.tensor_scalar(
                    out=dd, in0=S2a, scalar1=-4.0, scalar2=0.5, op0=Alu.add, op1=Alu.mult
                )
                nc.vector.scalar_tensor_tensor(
                    out=th, in0=dd, scalar=r1, in1=th, op0=Alu.mult, op1=Alu.add
                )

        def mask_mm_cde(t, half, th_mask):
            """Mask + matmuls + compaction + gather for one f-half.

            half=0: f [0:WH),  gids [0:4000),  capacity S4A, cand[:, 0:CWA]
            half=1: f [WH:WF), gids [4000:8000), capacity S4B, cand[:, CWA:CW]
            """
            g4, sc, cand, xh = t["g4"], t["sc"], t["cand"], t["xh"]
            s4 = S4A if half == 0 else S4B
            f0 = 0 if half == 0 else WH
            vc = sc[:, 48:67].bitcast(i16) if half == 0 else sc[:, 67:86].bitcast(i16)

            nc.vector.tensor_scalar(
                out=g4[:, 4000 * half : 4000 * (half + 1)],
                in0=g4[:, 4000 * half : 4000 * (half + 1)],
                scalar1=th_mask, scalar2=0.0,
                op0=Alu.subtract, op1=Alu.max,
            )
            pc = psum.tile([128, WH], f32, tag=f"pc{half}")
            g4v = g4[:].rearrange("p (f wr) -> p f wr", wr=16)
            for w in range(16):
                nc.tensor.matmul(
                    pc[:],
                    wt[:, bass.ts(w, 128)],
                    g4v[:, f0 : f0 + WH, w],
                    start=(w == 0),
                    stop=(w == 15),
                )

            nc.vector.tensor_scalar(out=maskv[:, 0:WH], in0=pc[:], scalar1=0.0, scalar2=None, op0=Alu.is_gt)
            nc.vector.scalar_tensor_tensor(
                out=v16[:, 0:WH], in0=maskv[:, 0:WH], scalar=1.0, in1=iot[:, f0 : f0 + WH],
                op0=Alu.mult, op1=Alu.mult,
            )
            nc.vector.tensor_tensor_scan(
                out=cum[:, 0:WH], data0=maskv[:, 0:WH], data1=maskv[:, 0:WH], initial=0.0,
                op0=Alu.add, op1=Alu.bypass,
            )
            nc.vector.tensor_tensor(out=cum[:, 0:WH], in0=cum[:, 0:WH], in1=maskv[:, 0:WH], op=Alu.mult)
            nc.vector.scalar_tensor_tensor(
                out=cum[:, 0:WH], in0=cum[:, 0:WH], scalar=float(s4) + 0.5, in1=cum[:, 0:WH],
                op0=Alu.is_le, op1=Alu.mult,
            )
            nc.vector.tensor_scalar(out=rank[:, 0:WH], in0=cum[:, 0:WH], scalar1=-1.0, scalar2=None, op0=Alu.add)
            nc.gpsimd.local_scatter(
                vc[:, 0:s4],
                v16[:, 0:WH],
                rank[:, 0:WH],
                channels=128,
                num_elems=s4,
                num_idxs=WH,
            )
            # group idx = (gid+1) - 1; scatter pads (0) map to sentinel group
            nc.vector.tensor_scalar(
                out=bneg[:, 0:s4], in0=vc[:, 0:s4], scalar1=0.5, scalar2=float(SENT) + 1.0,
                op0=Alu.is_lt, op1=Alu.mult,
            )
            nc.vector.scalar_tensor_tensor(
                out=bneg[:, 0:s4], in0=vc[:, 0:s4], scalar=1.0, in1=bneg[:, 0:s4],
                op0=Alu.mult, op1=Alu.add,
            )
            gix = v16[:, 0:s4]
            nc.vector.tensor_scalar(out=gix, in0=bneg[:, 0:s4], scalar1=-1.0, scalar2=None, op0=Alu.add)

            clo = 0 if half == 0 else CWA
            chi = CWA if half == 0 else CW
            nc.gpsimd.ap_gather(
                cand[:, clo:chi].rearrange("p (a d) -> p a d", d=4),
                xh[:].rearrange("p (a d) -> p a d", d=4),
                gix,
                channels=128,
                num_elems=DW // 4,
                d=4,
                num_idxs=16 * s4,
            )

        def phase_A(b):
            t = tiles[b]
            xh, g4, g64, sc = t["xh"], t["g4"], t["g64"], t["sc"]
            m_s = sc[:, 0:1]
            th = sc[:, 1:2]
            th_snapA = sc[:, 9:10]
            nc.vector.memset(xh[:, V:DW], NEG)
            xb = x[bass.ts(b, 128), :]
            for c in range(NCHUNK):
                xc = big.tile([128, CHUNK], f32, tag="xc", bufs=3)
                nc.sync.dma_start(xc[:], xb[:, bass.ts(c, CHUNK)])
                nc.scalar.activation(
                    xh[:, bass.ts(c, CHUNK)], xc[:], Act.Copy, bias=0.0, scale=1.0
                )
                eng = nc.gpsimd if c in GP_CHUNKS else nc.vector
                eng.tensor_reduce(
                    g4[:, bass.ts(c, G4C)],
                    xh[:, bass.ts(c, CHUNK)].rearrange("p (g k) -> p g k", k=4),
                    axis=Ax.X,
                    op=Alu.max,
                )
                nc.vector.tensor_reduce(
                    g64[:, bass.ts(c, G64C)],
                    g4[:, bass.ts(c, G4C)].rearrange("p (g k) -> p g k", k=16),
                    axis=Ax.X,
                    op=Alu.max,
                )
                if c == 10:
                    # progressive-newton warm start on g64 prefix [0:250]
                    nc.vector.tensor_reduce(m_s, g64[:, 0:250], axis=Ax.X, op=Alu.max)
                    nc.vector.tensor_scalar(
                        out=th, in0=m_s, scalar1=-2.0, scalar2=None, op0=Alu.add
                    )
                    newton_iters(t, 250, 5)
                elif c == 13:
                    newton_iters(t, 350, 2)
                elif c == 16:
                    newton_iters(t, 425, 2)
                    nc.vector.tensor_scalar(
                        out=th_snapA, in0=th, scalar1=-EPS_LB, scalar2=None, op0=Alu.add
                    )
                    # A-half mask/compact/gather runs under the remaining stream
                    mask_mm_cde(t, 0, th_snapA)

        def phase_tailB(b):
            t = tiles[b]
            sc = t["sc"]
            th = sc[:, 1:2]
            th_mB = sc[:, 10:11]
            nuF = sc[:, 11:12]
            newton_iters(t, WF, 2)
            nc.vector.tensor_scalar(out=th_mB, in0=th, scalar1=-EPS_LB, scalar2=None, op0=Alu.add)
            mask_mm_cde(t, 1, th_mB)
            # 3rd full iter refines theta for F; overlaps the B matmuls/compaction
            newton_iters(t, WF, 1)
            nc.vector.tensor_scalar(out=nuF, in0=th, scalar1=-1.0, scalar2=None, op0=Alu.mult)

        def phase_F(b):
            t = tiles[b]
            cand, sc = t["cand"], t["sc"]
            gA, gB = gtiles[b]
            th = sc[:, 1:2]
            nuF = sc[:, 11:12]
            r1 = sc[:, 7:8]
            one_t = sc[:, 0:1]
            S1 = sc[:, 40:41]
            S2 = sc[:, 41:42]
            S3 = sc[:, 42:43]
            dlt = sc[:, 12:13]
            e1 = sc[:, 13:14]
            tb_ = sc[:, 14:15]
            lo = sc[:, 15:16]

            # slice list: (start, width)
            slices = []
            for off in (0, 640, 1280, 1920):
                slices.append((off, min(640, CWA - off)))
            for off in (0, 640, 1280, 1920):
                slices.append((CWA + off, min(640, CWB - off)))

            for i, (o, w) in enumerate(slices):
                sl = slice(o, o + w)
                nc.scalar.activation(
                    wk[:, 0:w], cand[:, sl], Act.Relu, bias=nuF, scale=1.0,
                    accum_out=sc[:, 16 + i : 17 + i],
                )
                nc.scalar.activation(cand[:, sl], wk[:, 0:w], Act.Square,
                                     accum_out=sc[:, 24 + i : 25 + i])
                nc.vector.tensor_tensor(
                    out=cand[:, sl], in0=wk[:, 0:w], in1=cand[:, sl], op=Alu.mult
                )
                nc.vector.tensor_reduce(sc[:, 32 + i : 33 + i], cand[:, sl], axis=Ax.X, op=Alu.add)

            nc.vector.tensor_reduce(S1, sc[:, 16:24], axis=Ax.X, op=Alu.add)
            nc.vector.tensor_reduce(S2, sc[:, 24:32], axis=Ax.X, op=Alu.add)
            nc.vector.tensor_reduce(S3, sc[:, 32:40], axis=Ax.X, op=Alu.add)

            # Newton-corrected closed form
            nc.vector.reciprocal(r1, S1)
            nc.vector.tensor_scalar(
                out=dlt, in0=S2, scalar1=-4.0, scalar2=0.5, op0=Alu.add, op1=Alu.mult
            )
            nc.vector.tensor_tensor(out=dlt, in0=dlt, in1=r1, op=Alu.mult)

            # x[target] one-hot dot; one_t (==1.0) anchored on r1 so the static
            # scheduler cannot hoist these to the head of the queue
            nc.vector.tensor_scalar(
                out=one_t, in0=r1, scalar1=0.0, scalar2=1.0, op0=Alu.mult, op1=Alu.add
            )
            nc.vector.scalar_tensor_tensor(
                out=gB[:], in0=gA[:], scalar=one_t, in1=gB[:], op0=Alu.mult, op1=Alu.mult
            )
            nc.vector.tensor_reduce(xtg[:, b : b + 1], gB[:], axis=Ax.X, op=Alu.add)

            # S3c = S3 - 3*dlt*(S2 - S1*dlt);  loss = 4/3 + S3c/12 + th + dlt - x_t
            nc.vector.tensor_tensor(out=e1, in0=S1, in1=dlt, op=Alu.mult)
            nc.vector.tensor_tensor(out=e1, in0=S2, in1=e1, op=Alu.subtract)
            nc.vector.tensor_tensor(out=e1, in0=e1, in1=dlt, op=Alu.mult)
            nc.vector.scalar_tensor_tensor(
                out=S3, in0=e1, scalar=-3.0, in1=S3, op0=Alu.mult, op1=Alu.add
            )
            nc.vector.tensor_tensor(out=tb_, in0=th, in1=dlt, op=Alu.add)
            nc.vector.scalar_tensor_tensor(
                out=tb_, in0=S3, scalar=1.0 / 12.0, in1=tb_, op0=Alu.mult, op1=Alu.add
            )
            nc.vector.scalar_tensor_tensor(
                out=lo, in0=tb_, scalar=4.0 / 3.0, in1=xtg[:, b : b + 1],
                op0=Alu.add, op1=Alu.subtract,
            )
            nc.sync.dma_start(out[bass.ts(b, 128)], lo)

        nc.sync.dma_start(iot[:], iotd)
        nc.sync.dma_start(wt[:], wseld)
        phase_A(0)
        phase_tailB(0)
        phase_A(1)
        phase_tailB(1)
        phase_F(0)
        phase_F(1)

    nc.compile()
    return nc


def get_nc():
    if "nc" not in _nc_cache:
        _nc_cache["nc"] = _build_nc()
    return _nc_cache["nc"]


def make_in_maps(X, target):
    X = np.ascontiguousarray(np.asarray(X, dtype=np.float32))
    target = np.asarray(target).astype(np.int64)

    # wrapped gid+1 iota: iot[p, f] = 16*f + (p % 16) + 1
    pp, ff = np.meshgrid(np.arange(128), np.arange(WF), indexing="ij")
    iot = (16 * ff + (pp % 16) + 1).astype(np.int16)
    # residue-selection matrices: wsel[p, w, n] = 1 if n == 16*(p//16) + w
    wsel = np.zeros((128, 16, 128), np.float32)
    for w in range(16):
        for p in range(128):
            wsel[p, w, 16 * (p // 16) + w] = 1.0
    wsel = wsel.reshape(128, 16 * 128).astype(np.float16)

    in_maps = []
    for k in range(N_CORES):
        Xk = X[k * ROWS : (k + 1) * ROWS]
        tk = target[k * ROWS : (k + 1) * ROWS]
        ohk = np.zeros((ROWS, 64), np.float32)
        ohk[np.arange(ROWS), (tk % 64).astype(np.int64)] = 1.0
        tblk = np.zeros((128, 16), np.int16)
        for c in range(4):
            rows = np.arange(64)
            vals = (rows * (V // 64) + (tk[64 * c + rows] // 64)).astype(np.int16)
            w = np.zeros((16, 4), np.int16)
            w[rows % 16, rows // 16] = vals
            tblk[:, 4 * c : 4 * (c + 1)] = np.tile(w, (8, 1))
        in_maps.append({"x": Xk, "oh": ohk, "tbl": tblk, "iot": iot, "wsel": wsel})
    return in_maps


def kernel(X, target):
    from concourse.bass_utils import run_bass_kernel_spmd

    nc = get_nc()
    in_maps = make_in_maps(X, target)
    res = run_bass_kernel_spmd(nc, in_maps, core_ids=list(range(N_CORES)))
    loss = np.concatenate([r["loss"] for r in res.results]).astype(np.float32)
    return loss


# revision 10
# speedup vs baseline: 1.3190x; 1.3028x over previous
"""Exact entmax-1.5 loss kernel for Trainium2 (8 NeuronCores, data-parallel over rows).

Algorithm (per row of X [N=2048, V=32000] f32):
  The entmax-1.5 threshold tau* solves  sum_j relu(X_j/2 - tau)^2 = 1.
  In X-units (theta = 2*tau):            sum_j relu(X_j - theta)^2 = 4.
  f(theta) is convex decreasing; Newton from a lower bound converges
  monotonically from below - no sort needed.

  v4 pipeline per 128-row block (fp16 payloads / bf16 bounds):
    A. Stream X in 16 f32 column chunks (DMA from SP queue); scalar converts
       to resident fp16 xh; vector builds group-of-4 maxes g4 bf16 and
       (per 4-chunk quarter) group-of-64 maxes g64 bf16.  A 4-iteration
       partial Newton on the first 3 quarters of g64 warm-starts theta while
       the last quarter still streams.
    B. 3 full all-vector Newton iterations on g64 -> theta_lb.
    C. mask = relu(g4 - theta_lb) in place; 16 accumulating 500-col bf16
       matmuls with residue-selection weights -> per-cluster group activity
       in wrapped [128,500] PSUM layout.
    D. Compaction: iota*mask, scan -> ranks, local_scatter -> 64 gids per
       partition, sentinel fixup (f16 mask/cum scratch).
    E. Two ap_gathers (512 idxs each) pull 4-fp16 payloads of the
       cluster-union candidate groups from xh -> cand [128,4096] fp16.
    F. Exact Newton (2 iters) + final stats on cand, column-split between
       scalar (activation accumulators) and vector engines;
       loss = 4/3 + S3/12 + theta*S2/4 - X[target] (X[target] via up-front
       dma_gather + one-hot dot, f32 exact).

  Emission order software-pipelines the two blocks: A0, B0..E0, A1, F0,
  B1..E1, F1 so block 1 streams under block 0's tail and the two ~30us
  gpsimd gathers overlap other engines' work.

Host wrapper shards rows 256-per-core across 8 cores, no collectives.
"""
import numpy as np
from contextlib import ExitStack

N, V = 2048, 32000
N_CORES = 8
ROWS = N // N_CORES          # 256 rows per core
CHUNK = 2000
NCHUNK = V // CHUNK          # 16
NG4 = V // 4                 # 8000 groups of 4
WF = 500                     # wrapped cols: group = 16*F + (p % 16)
PART_ITERS = 4               # partial-newton iters on first 375 g64 cols
FULL_ITERS = 2
EXACT_ITERS = 2
EPS_LB = 2e-2                # X-units safety margin (covers bf16 bound noise)
S4 = 60                      # per-partition capacity of compacted group ids
KU = 16 * S4                 # 1024 union groups per 16-partition cluster
CW = 4 * KU                  # 4096 compact width (fp16)
GPAD = 4
NGP = NG4 + GPAD             # 8008 groups incl. sentinel pad
DW = 4 * NGP                 # 32032 fp16 X width
SENT = NG4 + 2               # sentinel group id for scatter pads
HF = 2560                    # F-phase split: scalar [0:HF], vector [HF:CW]
NEG = -60000.0               # fp16-safe -inf substitute

_nc_cache = {}


def _build_nc():
    import concourse.bass as bass
    import concourse.bacc as bacc
    import concourse.tile as tile
    from concourse import mybir

    f32 = mybir.dt.float32
    f16 = mybir.dt.float16
    bf16 = mybir.dt.bfloat16
    i16 = mybir.dt.int16
    Alu = mybir.AluOpType
    Act = mybir.ActivationFunctionType
    Ax = mybir.AxisListType

    nc = bacc.Bacc("TRN2", target_bir_lowering=False, debug=False)
    x = nc.dram_tensor("x", [ROWS, V], f32, kind="ExternalInput").ap()
    oh = nc.dram_tensor("oh", [ROWS, 64], f32, kind="ExternalInput").ap()
    tbl = nc.dram_tensor("tbl", [128, 16], i16, kind="ExternalInput").ap()
    iotd = nc.dram_tensor("iot", [128, WF], f32, kind="ExternalInput").ap()
    wseld = nc.dram_tensor("wsel", [128, 16 * 128], bf16, kind="ExternalInput").ap()
    out = nc.dram_tensor("loss", [ROWS], f32, kind="ExternalOutput").ap()

    with tile.TileContext(nc) as tc, ExitStack() as ctx:
        const = ctx.enter_context(tc.tile_pool(name="const", bufs=1))
        big = ctx.enter_context(tc.tile_pool(name="big", bufs=1))
        psum = ctx.enter_context(tc.tile_pool(name="psum", bufs=2, space="PSUM"))

        iot = const.tile([128, WF], f32, tag="iot")
        wt = const.tile([128, 16 * 128], bf16, tag="wsel")
        xtg = const.tile([128, 2], f32, tag="xtg")
        tbl_t = const.tile([128, 16], i16, tag="tbl")

        nc.sync.dma_start(tbl_t[:], tbl[:, :])

        # ---- x[target] DMA gathers for both blocks, up front (gpsimd mlp lib).
        # The one-hot dot products are deferred to phase F to keep the vector
        # queue free for streaming.
        gtiles = []
        for b in range(2):
            gA = big.tile([128, 64], f32, tag="gA", bufs=2)
            gB = big.tile([128, 64], f32, tag="gB", bufs=2)
            for half, gdst in ((0, gA), (1, gB)):
                c = 2 * b + half
                src = x[64 * c : 64 * (c + 1), :].rearrange(
                    "r (bk e) -> (r bk) e", e=64
                )
                nc.gpsimd.dma_gather(
                    gdst.rearrange("p (one e) -> p one e", one=1),
                    src,
                    tbl_t[:, 4 * c : 4 * (c + 1)],
                    num_idxs=64,
                    num_idxs_reg=64,
                    elem_size=64,
                )
            nc.gpsimd.dma_start(gA[64:128, :], gB[0:64, :])
            # after the combine, gB is reloaded with the one-hot rows; issued
            # from the gpsimd queue so the dependency cannot block sync's
            # in-order chunk-trigger stream
            nc.gpsimd.dma_start(gB[:], oh[bass.ts(b, 128), :])
            gtiles.append((gA, gB))

        # dummy 16-idx ap_gather: pre-loads the Q7 ap_gather library while
        # block 0 is still streaming, so the real gathers start instantly
        sc0_w = big.tile([128, 80], f32, tag="warm", bufs=1)
        wix = sc0_w[:, 79:80].bitcast(i16)[:, 0:1]
        nc.vector.memset(wix, 0)
        # reads gB of block 1 (last mlp-gather output) so this runs after the
        # mlp library is done with it, keeping ap_gather resident for APG0
        nc.gpsimd.ap_gather(
            sc0_w[:, 0:64].rearrange("p (a d) -> p a d", d=4),
            gtiles[1][1][:, 0:16].rearrange("p (a d) -> p a d", d=4),
            wix,
            channels=128,
            num_elems=4,
            d=4,
            num_idxs=16,
        )

        # per-block tiles (tag rotation gives block-alternating buffers)
        tiles = []
        for b in range(2):
            t = {}
            t["xh"] = big.tile([128, DW], f16, name="xh", tag="xh", bufs=2)
            t["g4"] = big.tile([128, NG4], bf16, name="g4", tag="g4", bufs=2)
            t["cand"] = big.tile([128, CW], f16, name="cand", tag="cand", bufs=2)
            t["wk"] = big.tile([128, HF], f16, name="wk", tag="wk", bufs=1)
            t["sc"] = big.tile([128, 128], f32, name="sc", tag="sc", bufs=2)
            t["scr"] = big.tile([128, 768], f32, name="scr", tag="scr", bufs=2)
            tiles.append(t)

        def g64_of(t):
            return t["scr"][:, 512:768].bitcast(bf16)

        def newton_iters(t, g, width, iters):
            sc, scr = t["sc"], t["scr"]
            th = sc[:, 1:2]
            nuB = sc[:, 2:3]
            S1a = sc[:, 3:4]
            S2a = sc[:, 4:5]
            r1 = sc[:, 7:8]
            dd = sc[:, 8:9]
            uB = scr[:, 0:256].bitcast(bf16)
            for _ in range(iters):
                nc.vector.tensor_scalar(out=nuB, in0=th, scalar1=-1.0, scalar2=None, op0=Alu.mult)
                nc.scalar.activation(
                    uB[:, 0:width], g[:, 0:width], Act.Relu, bias=nuB, scale=1.0,
                    accum_out=S1a,
                )
                nc.scalar.activation(uB[:, 0:width], uB[:, 0:width], Act.Square, accum_out=S2a)
                nc.vector.reciprocal(r1, S1a)
                nc.vector.tensor_scalar(
                    out=dd, in0=S2a, scalar1=-4.0, scalar2=0.5, op0=Alu.add, op1=Alu.mult
                )
                nc.vector.scalar_tensor_tensor(
                    out=th, in0=dd, scalar=r1, in1=th, op0=Alu.mult, op1=Alu.add
                )

        def phase_A(b):
            t = tiles[b]
            xh, g4, sc = t["xh"], t["g4"], t["sc"]
            g64 = g64_of(t)
            m_s = sc[:, 0:1]
            th = sc[:, 1:2]
            nc.vector.memset(xh[:, V:DW], NEG)
            xb = x[bass.ts(b, 128), :]
            for c in range(NCHUNK):
                xc = big.tile([128, CHUNK], f32, tag="xc", bufs=2)
                nc.sync.dma_start(xc[:], xb[:, bass.ts(c, CHUNK)])
                nc.scalar.activation(
                    xh[:, bass.ts(c, CHUNK)], xc[:], Act.Copy, bias=0.0, scale=1.0
                )
                nc.vector.tensor_reduce(
                    g4[:, bass.ts(c, CHUNK // 4)],
                    xh[:, bass.ts(c, CHUNK)].rearrange("p (g k) -> p g k", k=4),
                    axis=Ax.X,
                    op=Alu.max,
                )
                if c % 4 == 3:
                    q = c // 4
                    nc.vector.tensor_reduce(
                        g64[:, bass.ts(q, 125)],
                        g4[:, bass.ts(q, 2000)].rearrange("p (g k) -> p g k", k=16),
                        axis=Ax.X,
                        op=Alu.max,
                    )
                if c == 11:
                    # partial-newton warm start on quarters 0-2 (375 cols)
                    nc.vector.tensor_reduce(m_s, g64[:, 0:375], axis=Ax.X, op=Alu.max)
                    nc.vector.tensor_scalar(
                        out=th, in0=m_s, scalar1=-2.0, scalar2=None, op0=Alu.add
                    )
                    newton_iters(t, g64, 375, PART_ITERS)

        def phase_BCDE(b):
            t = tiles[b]
            xh, g4, sc, scr = t["xh"], t["g4"], t["sc"], t["scr"]
            g64 = g64_of(t)

            th = sc[:, 1:2]
            nu = sc[:, 2:3]
            vcomp = sc[:, 16:48].bitcast(i16)      # [128, 64] i16
            bneg = sc[:, 48:112]                    # [128, 64] f32

            maskv = scr[:, 0:250].bitcast(f16)      # [128, 500] f16
            cum = scr[:, 250:500].bitcast(f16)      # [128, 500] f16
            v16 = scr[:, 512:768].bitcast(i16)      # [128, 512] i16
            rank = scr[:, 0:256].bitcast(i16)       # [128, 512] i16, reuses maskv

            # ---- B: finish the G2 newton on the full 500 cols ----
            newton_iters(t, g64, WF, FULL_ITERS)
            nc.vector.tensor_scalar(out=th, in0=th, scalar1=-EPS_LB, scalar2=None, op0=Alu.add)
            nc.vector.tensor_scalar(out=nu, in0=th, scalar1=-1.0, scalar2=None, op0=Alu.mult)

            # ---- C: candidate mask (in place on g4) + residue matmuls ----
            nc.vector.tensor_scalar(
                out=g4[:], in0=g4[:], scalar1=th, scalar2=0.0,
                op0=Alu.subtract, op1=Alu.max,
            )
            pc = psum.tile([128, WF], f32, tag="pc")
            g4v = g4[:].rearrange("p (f wr) -> p f wr", wr=16)
            for w in range(16):
                nc.tensor.matmul(
                    pc[:],
                    wt[:, bass.ts(w, 128)],
                    g4v[:, :, w],
                    start=(w == 0),
                    stop=(w == 15),
                )

            # ---- D: compaction ----
            nc.vector.tensor_scalar(out=maskv, in0=pc[:], scalar1=0.0, scalar2=None, op0=Alu.is_gt)
            nc.vector.scalar_tensor_tensor(
                out=v16[:, 0:WF], in0=maskv, scalar=1.0, in1=iot[:],
                op0=Alu.mult, op1=Alu.mult,
            )
            nc.vector.tensor_tensor_scan(
                out=cum, data0=maskv, data1=maskv, initial=0.0,
                op0=Alu.add, op1=Alu.bypass,
            )
            nc.vector.tensor_tensor(out=cum, in0=cum, in1=maskv, op=Alu.mult)
            nc.vector.scalar_tensor_tensor(
                out=cum, in0=cum, scalar=float(S4) + 0.5, in1=cum,
                op0=Alu.is_le, op1=Alu.mult,
            )
            nc.vector.tensor_scalar(out=rank[:, 0:WF], in0=cum, scalar1=-1.0, scalar2=None, op0=Alu.add)
            nc.gpsimd.local_scatter(
                vcomp[:, 0:S4],
                v16[:, 0:WF],
                rank[:, 0:WF],
                channels=128,
                num_elems=S4,
                num_idxs=WF,
            )
            # group idx = (gid+1) - 1; scatter pads (0) map to sentinel group
            nc.vector.tensor_scalar(
                out=bneg[:, 0:S4], in0=vcomp[:, 0:S4], scalar1=0.5, scalar2=float(SENT) + 1.0,
                op0=Alu.is_lt, op1=Alu.mult,
            )
            nc.vector.scalar_tensor_tensor(
                out=bneg[:, 0:S4], in0=vcomp[:, 0:S4], scalar=1.0, in1=bneg[:, 0:S4],
                op0=Alu.mult, op1=Alu.add,
            )
            gix = v16[:, 0:S4]
            nc.vector.tensor_scalar(out=gix, in0=bneg[:, 0:S4], scalar1=-1.0, scalar2=None, op0=Alu.add)

            # ---- E: gather candidate payloads from xh (two halves) ----
            cand = t["cand"]
            for lo_s, hi_s in ((0, 20), (20, 40), (40, 60)):
                nc.gpsimd.ap_gather(
                    cand[:, 64 * lo_s : 64 * hi_s].rearrange("p (a d) -> p a d", d=4),
                    xh[:].rearrange("p (a d) -> p a d", d=4),
                    gix[:, lo_s:hi_s],
                    channels=128,
                    num_elems=DW // 4,
                    d=4,
                    num_idxs=16 * (hi_s - lo_s),
                )

        def phase_F(b):
            t = tiles[b]
            cand, wk, sc = t["cand"], t["wk"], t["sc"]
            gA, gB = gtiles[b]
            th = sc[:, 1:2]
            nu = sc[:, 2:3]
            r1 = sc[:, 7:8]
            one_t = sc[:, 0:1]   # reuses m_s slot (dead after phase A)
            S1s = sc[:, 112:115]
            S2s = sc[:, 115:118]
            S3s = sc[:, 118:121]
            S1 = sc[:, 121:122]
            S2 = sc[:, 122:123]
            S3 = sc[:, 123:124]
            dlt = sc[:, 124:125]
            e1 = sc[:, 125:126]
            tb_ = sc[:, 126:127]
            lo = sc[:, 15:16]

            # Taylor-fused stats: ONE pass over cand computing S1,S2 (scalar
            # activation accumulators) and S3 (vector mult+reduce) at theta0 =
            # th (the newton lower bound incl. -EPS), then a Newton-corrected
            # closed form.  d loss/d th = 1 - S2/4 -> 0 at theta*, so the
            # single correction reaches ~1e-4 accuracy.
            for i in range(3):
                sl = slice(1280 * i, 1280 * (i + 1))
                wkh = wk[:, 1280 * (i % 2) : 1280 * (i % 2) + 1280]
                nc.scalar.activation(
                    wkh, cand[:, sl], Act.Relu, bias=nu, scale=1.0,
                    accum_out=S1s[:, i : i + 1],
                )
                nc.scalar.activation(cand[:, sl], wkh, Act.Square,
                                     accum_out=S2s[:, i : i + 1])
                nc.vector.tensor_tensor(
                    out=cand[:, sl], in0=wkh, in1=cand[:, sl], op=Alu.mult
                )
                nc.vector.tensor_reduce(S3s[:, i : i + 1], cand[:, sl], axis=Ax.X, op=Alu.add)

            nc.vector.tensor_reduce(S1, S1s, axis=Ax.X, op=Alu.add)
            nc.vector.tensor_reduce(S2, S2s, axis=Ax.X, op=Alu.add)
            nc.vector.tensor_reduce(S3, S3s, axis=Ax.X, op=Alu.add)

            nc.vector.reciprocal(r1, S1)
            nc.vector.tensor_scalar(
                out=dlt, in0=S2, scalar1=-4.0, scalar2=0.5, op0=Alu.add, op1=Alu.mult
            )
            nc.vector.tensor_tensor(out=dlt, in0=dlt, in1=r1, op=Alu.mult)

            # x[target] one-hot dot; one_t (==1.0) depends on the newton chain so
            # the static scheduler cannot hoist these to the head of the queue
            nc.vector.tensor_scalar(
                out=one_t, in0=r1, scalar1=0.0, scalar2=1.0, op0=Alu.mult, op1=Alu.add
            )
            nc.vector.scalar_tensor_tensor(
                out=gB[:], in0=gA[:], scalar=one_t, in1=gB[:], op0=Alu.mult, op1=Alu.mult
            )
            nc.vector.tensor_reduce(xtg[:, b : b + 1], gB[:], axis=Ax.X, op=Alu.add)

            # S3c = S3 - 3*dlt*(S2 - S1*dlt); loss = 4/3 + S3c/12 + th+dlt - x_t
            nc.vector.tensor_tensor(out=e1, in0=S1, in1=dlt, op=Alu.mult)
            nc.vector.tensor_tensor(out=e1, in0=S2, in1=e1, op=Alu.subtract)
            nc.vector.tensor_tensor(out=e1, in0=e1, in1=dlt, op=Alu.mult)
            nc.vector.scalar_tensor_tensor(
                out=S3, in0=e1, scalar=-3.0, in1=S3, op0=Alu.mult, op1=Alu.add
            )
            nc.vector.tensor_tensor(out=tb_, in0=th, in1=dlt, op=Alu.add)
            nc.vector.scalar_tensor_tensor(
                out=tb_, in0=S3, scalar=1.0 / 12.0, in1=tb_, op0=Alu.mult, op1=Alu.add
            )
            nc.vector.scalar_tensor_tensor(
                out=lo, in0=tb_, scalar=4.0 / 3.0, in1=xtg[:, b : b + 1],
                op0=Alu.add, op1=Alu.subtract,
            )
            nc.sync.dma_start(out[bass.ts(b, 128)], lo)

        phase_A(0)
        nc.sync.dma_start(iot[:], iotd)
        nc.sync.dma_start(wt[:], wseld)
        phase_BCDE(0)
        phase_A(1)
        phase_F(0)
        phase_BCDE(1)
        phase_F(1)

    nc.compile()
    return nc


def get_nc():
    if "nc" not in _nc_cache:
        _nc_cache["nc"] = _build_nc()
    return _nc_cache["nc"]


def make_in_maps(X, target):
    import ml_dtypes

    X = np.ascontiguousarray(np.asarray(X, dtype=np.float32))
    target = np.asarray(target).astype(np.int64)

    # wrapped gid+1 iota: iot[p, f] = 16*f + (p % 16) + 1
    pp, ff = np.meshgrid(np.arange(128), np.arange(WF), indexing="ij")
    iot = (16 * ff + (pp % 16) + 1).astype(np.float32)
    # residue-selection matrices: wsel[p, w, n] = 1 if n == 16*(p//16) + w
    wsel = np.zeros((128, 16, 128), np.float32)
    for w in range(16):
        for p in range(128):
            wsel[p, w, 16 * (p // 16) + w] = 1.0
    wsel = wsel.reshape(128, 16 * 128).astype(ml_dtypes.bfloat16)

    in_maps = []
    for k in range(N_CORES):
        Xk = X[k * ROWS : (k + 1) * ROWS]
        tk = target[k * ROWS : (k + 1) * ROWS]
        ohk = np.zeros((ROWS, 64), np.float32)
        ohk[np.arange(ROWS), (tk % 64).astype(np.int64)] = 1.0
        tblk = np.zeros((128, 16), np.int16)
        for c in range(4):
            rows = np.arange(64)
            vals = (rows * (V // 64) + (tk[64 * c + rows] // 64)).astype(np.int16)
            w = np.zeros((16, 4), np.int16)
            w[rows % 16, rows // 16] = vals
            tblk[:, 4 * c : 4 * (c + 1)] = np.tile(w, (8, 1))
        in_maps.append({"x": Xk, "oh": ohk, "tbl": tblk, "iot": iot, "wsel": wsel})
    return in_maps


def kernel(X, target):
    from concourse.bass_utils import run_bass_kernel_spmd

    nc = get_nc()
    in_maps = make_in_maps(X, target)
    res = run_bass_kernel_spmd(nc, in_maps, core_ids=list(range(N_CORES)))
    loss = np.concatenate([r["loss"] for r in res.results]).astype(np.float32)
    return loss



# revision 11
# speedup vs baseline: 1.3239x; 1.0038x over previous
"""Exact entmax-1.5 loss kernel for Trainium2 (8 NeuronCores, data-parallel over rows).

Algorithm (per row of X [N=2048, V=32000] f32):
  The entmax-1.5 threshold tau* solves  sum_j relu(X_j/2 - tau)^2 = 1.
  In X-units (theta = 2*tau):            sum_j relu(X_j - theta)^2 = 4.
  f(theta) is convex decreasing; Newton from a lower bound converges
  monotonically from below - no sort needed.

  v4 pipeline per 128-row block (fp16 payloads / bf16 bounds):
    A. Stream X in 16 f32 column chunks (DMA from SP queue); scalar converts
       to resident fp16 xh; vector builds group-of-4 maxes g4 bf16 and
       (per 4-chunk quarter) group-of-64 maxes g64 bf16.  A 4-iteration
       partial Newton on the first 3 quarters of g64 warm-starts theta while
       the last quarter still streams.
    B. 3 full all-vector Newton iterations on g64 -> theta_lb.
    C. mask = relu(g4 - theta_lb) in place; 16 accumulating 500-col bf16
       matmuls with residue-selection weights -> per-cluster group activity
       in wrapped [128,500] PSUM layout.
    D. Compaction: iota*mask, scan -> ranks, local_scatter -> 64 gids per
       partition, sentinel fixup (f16 mask/cum scratch).
    E. Two ap_gathers (512 idxs each) pull 4-fp16 payloads of the
       cluster-union candidate groups from xh -> cand [128,4096] fp16.
    F. Exact Newton (2 iters) + final stats on cand, column-split between
       scalar (activation accumulators) and vector engines;
       loss = 4/3 + S3/12 + theta*S2/4 - X[target] (X[target] via up-front
       dma_gather + one-hot dot, f32 exact).

  Emission order software-pipelines the two blocks: A0, B0..E0, A1, F0,
  B1..E1, F1 so block 1 streams under block 0's tail and the two ~30us
  gpsimd gathers overlap other engines' work.

Host wrapper shards rows 256-per-core across 8 cores, no collectives.
"""
import numpy as np
from contextlib import ExitStack

N, V = 2048, 32000
N_CORES = 8
ROWS = N // N_CORES          # 256 rows per core
CHUNK = 2000
NCHUNK = V // CHUNK          # 16
NG4 = V // 4                 # 8000 groups of 4
WF = 500                     # wrapped cols: group = 16*F + (p % 16)
PART_ITERS = 4               # partial-newton iters on first 375 g64 cols
FULL_ITERS = 2
EXACT_ITERS = 2
EPS_LB = 2e-2                # X-units safety margin (covers bf16 bound noise)
S4 = 60                      # per-partition capacity of compacted group ids
KU = 16 * S4                 # 1024 union groups per 16-partition cluster
CW = 4 * KU                  # 4096 compact width (fp16)
GPAD = 4
NGP = NG4 + GPAD             # 8008 groups incl. sentinel pad
DW = 4 * NGP                 # 32032 fp16 X width
SENT = NG4 + 2               # sentinel group id for scatter pads
HF = 2560                    # F-phase split: scalar [0:HF], vector [HF:CW]
NEG = -60000.0               # fp16-safe -inf substitute

_nc_cache = {}


def _build_nc():
    import concourse.bass as bass
    import concourse.bacc as bacc
    import concourse.tile as tile
    from concourse import mybir

    f32 = mybir.dt.float32
    f16 = mybir.dt.float16
    bf16 = mybir.dt.bfloat16
    i16 = mybir.dt.int16
    Alu = mybir.AluOpType
    Act = mybir.ActivationFunctionType
    Ax = mybir.AxisListType

    nc = bacc.Bacc("TRN2", target_bir_lowering=False, debug=False)
    x = nc.dram_tensor("x", [ROWS, V], f32, kind="ExternalInput").ap()
    oh = nc.dram_tensor("oh", [ROWS, 64], f32, kind="ExternalInput").ap()
    tbl = nc.dram_tensor("tbl", [128, 16], i16, kind="ExternalInput").ap()
    iotd = nc.dram_tensor("iot", [128, WF], f32, kind="ExternalInput").ap()
    wseld = nc.dram_tensor("wsel", [128, 16 * 128], bf16, kind="ExternalInput").ap()
    out = nc.dram_tensor("loss", [ROWS], f32, kind="ExternalOutput").ap()

    with tile.TileContext(nc) as tc, ExitStack() as ctx:
        const = ctx.enter_context(tc.tile_pool(name="const", bufs=1))
        big = ctx.enter_context(tc.tile_pool(name="big", bufs=1))
        psum = ctx.enter_context(tc.tile_pool(name="psum", bufs=2, space="PSUM"))

        iot = const.tile([128, WF], f32, tag="iot")
        wt = const.tile([128, 16 * 128], bf16, tag="wsel")
        xtg = const.tile([128, 2], f32, tag="xtg")
        tbl_t = const.tile([128, 16], i16, tag="tbl")

        nc.sync.dma_start(tbl_t[:], tbl[:, :])

        # ---- x[target] DMA gathers for both blocks, up front (gpsimd mlp lib).
        # The one-hot dot products are deferred to phase F to keep the vector
        # queue free for streaming.
        gtiles = []
        for b in range(2):
            gA = big.tile([128, 64], f32, tag="gA", bufs=2)
            gB = big.tile([128, 64], f32, tag="gB", bufs=2)
            for half, gdst in ((0, gA), (1, gB)):
                c = 2 * b + half
                src = x[64 * c : 64 * (c + 1), :].rearrange(
                    "r (bk e) -> (r bk) e", e=64
                )
                nc.gpsimd.dma_gather(
                    gdst.rearrange("p (one e) -> p one e", one=1),
                    src,
                    tbl_t[:, 4 * c : 4 * (c + 1)],
                    num_idxs=64,
                    num_idxs_reg=64,
                    elem_size=64,
                )
            nc.gpsimd.dma_start(gA[64:128, :], gB[0:64, :])
            # after the combine, gB is reloaded with the one-hot rows; issued
            # from the gpsimd queue so the dependency cannot block sync's
            # in-order chunk-trigger stream
            nc.gpsimd.dma_start(gB[:], oh[bass.ts(b, 128), :])
            gtiles.append((gA, gB))

        # dummy 16-idx ap_gather: pre-loads the Q7 ap_gather library while
        # block 0 is still streaming, so the real gathers start instantly
        sc0_w = big.tile([128, 80], f32, tag="warm", bufs=1)
        wix = sc0_w[:, 79:80].bitcast(i16)[:, 0:1]
        nc.vector.memset(wix, 0)
        # reads gB of block 1 (last mlp-gather output) so this runs after the
        # mlp library is done with it, keeping ap_gather resident for APG0
        nc.gpsimd.ap_gather(
            sc0_w[:, 0:64].rearrange("p (a d) -> p a d", d=4),
            gtiles[1][1][:, 0:16].rearrange("p (a d) -> p a d", d=4),
            wix,
            channels=128,
            num_elems=4,
            d=4,
            num_idxs=16,
        )

        # per-block tiles (tag rotation gives block-alternating buffers)
        tiles = []
        for b in range(2):
            t = {}
            t["xh"] = big.tile([128, DW], f16, name="xh", tag="xh", bufs=2)
            t["g4"] = big.tile([128, NG4], bf16, name="g4", tag="g4", bufs=2)
            t["cand"] = big.tile([128, CW], f16, name="cand", tag="cand", bufs=2)
            t["wk"] = big.tile([128, HF], f16, name="wk", tag="wk", bufs=1)
            t["sc"] = big.tile([128, 128], f32, name="sc", tag="sc", bufs=2)
            t["scr"] = big.tile([128, 768], f32, name="scr", tag="scr", bufs=2)
            tiles.append(t)

        def g64_of(t):
            return t["scr"][:, 512:768].bitcast(bf16)

        def newton_iters(t, g, width, iters):
            sc, scr = t["sc"], t["scr"]
            th = sc[:, 1:2]
            nuB = sc[:, 2:3]
            S1a = sc[:, 3:4]
            S2a = sc[:, 4:5]
            r1 = sc[:, 7:8]
            dd = sc[:, 8:9]
            uB = scr[:, 0:256].bitcast(bf16)
            for _ in range(iters):
                nc.vector.tensor_scalar(out=nuB, in0=th, scalar1=-1.0, scalar2=None, op0=Alu.mult)
                nc.scalar.activation(
                    uB[:, 0:width], g[:, 0:width], Act.Relu, bias=nuB, scale=1.0,
                    accum_out=S1a,
                )
                nc.scalar.activation(uB[:, 0:width], uB[:, 0:width], Act.Square, accum_out=S2a)
                nc.vector.reciprocal(r1, S1a)
                nc.vector.tensor_scalar(
                    out=dd, in0=S2a, scalar1=-4.0, scalar2=0.5, op0=Alu.add, op1=Alu.mult
                )
                nc.vector.scalar_tensor_tensor(
                    out=th, in0=dd, scalar=r1, in1=th, op0=Alu.mult, op1=Alu.add
                )

        def phase_A(b):
            t = tiles[b]
            xh, g4, sc = t["xh"], t["g4"], t["sc"]
            g64 = g64_of(t)
            m_s = sc[:, 0:1]
            th = sc[:, 1:2]
            nc.vector.memset(xh[:, V:DW], NEG)
            xb = x[bass.ts(b, 128), :]
            for c in range(NCHUNK):
                xc = big.tile([128, CHUNK], f32, tag="xc", bufs=2)
                nc.sync.dma_start(xc[:], xb[:, bass.ts(c, CHUNK)])
                nc.scalar.activation(
                    xh[:, bass.ts(c, CHUNK)], xc[:], Act.Copy, bias=0.0, scale=1.0
                )
                nc.vector.tensor_reduce(
                    g4[:, bass.ts(c, CHUNK // 4)],
                    xh[:, bass.ts(c, CHUNK)].rearrange("p (g k) -> p g k", k=4),
                    axis=Ax.X,
                    op=Alu.max,
                )
                if c % 4 == 3:
                    q = c // 4
                    nc.vector.tensor_reduce(
                        g64[:, bass.ts(q, 125)],
                        g4[:, bass.ts(q, 2000)].rearrange("p (g k) -> p g k", k=16),
                        axis=Ax.X,
                        op=Alu.max,
                    )
                if c == 11:
                    # partial-newton warm start on quarters 0-2 (375 cols)
                    nc.vector.tensor_reduce(m_s, g64[:, 0:375], axis=Ax.X, op=Alu.max)
                    nc.vector.tensor_scalar(
                        out=th, in0=m_s, scalar1=-2.0, scalar2=None, op0=Alu.add
                    )
                    newton_iters(t, g64, 375, PART_ITERS)

        def phase_BCDE(b):
            t = tiles[b]
            xh, g4, sc, scr = t["xh"], t["g4"], t["sc"], t["scr"]
            g64 = g64_of(t)

            th = sc[:, 1:2]
            nu = sc[:, 2:3]
            vcomp = sc[:, 16:48].bitcast(i16)      # [128, 64] i16
            bneg = sc[:, 48:112]                    # [128, 64] f32

            maskv = scr[:, 0:250].bitcast(f16)      # [128, 500] f16
            cum = scr[:, 250:500].bitcast(f16)      # [128, 500] f16
            v16 = scr[:, 512:768].bitcast(i16)      # [128, 512] i16
            rank = scr[:, 0:256].bitcast(i16)       # [128, 512] i16, reuses maskv

            # ---- B: finish the G2 newton on the full 500 cols ----
            newton_iters(t, g64, WF, FULL_ITERS)
            nc.vector.tensor_scalar(out=th, in0=th, scalar1=-EPS_LB, scalar2=None, op0=Alu.add)
            nc.vector.tensor_scalar(out=nu, in0=th, scalar1=-1.0, scalar2=None, op0=Alu.mult)

            # ---- C: candidate mask (in place on g4) + residue matmuls ----
            nc.vector.tensor_scalar(
                out=g4[:], in0=g4[:], scalar1=th, scalar2=0.0,
                op0=Alu.subtract, op1=Alu.max,
            )
            pc = psum.tile([128, WF], f32, tag="pc")
            g4v = g4[:].rearrange("p (f wr) -> p f wr", wr=16)
            for w in range(16):
                nc.tensor.matmul(
                    pc[:],
                    wt[:, bass.ts(w, 128)],
                    g4v[:, :, w],
                    start=(w == 0),
                    stop=(w == 15),
                )

            # ---- D: compaction ----
            nc.vector.tensor_scalar(out=maskv, in0=pc[:], scalar1=0.0, scalar2=None, op0=Alu.is_gt)
            nc.vector.scalar_tensor_tensor(
                out=v16[:, 0:WF], in0=maskv, scalar=1.0, in1=iot[:],
                op0=Alu.mult, op1=Alu.mult,
            )
            nc.vector.tensor_tensor_scan(
                out=cum, data0=maskv, data1=maskv, initial=0.0,
                op0=Alu.add, op1=Alu.bypass,
            )
            nc.vector.tensor_tensor(out=cum, in0=cum, in1=maskv, op=Alu.mult)
            nc.vector.scalar_tensor_tensor(
                out=cum, in0=cum, scalar=float(S4) + 0.5, in1=cum,
                op0=Alu.is_le, op1=Alu.mult,
            )
            nc.vector.tensor_scalar(out=rank[:, 0:WF], in0=cum, scalar1=-1.0, scalar2=None, op0=Alu.add)
            nc.gpsimd.local_scatter(
                vcomp[:, 0:S4],
                v16[:, 0:WF],
                rank[:, 0:WF],
                channels=128,
                num_elems=S4,
                num_idxs=WF,
            )
            # group idx = (gid+1) - 1; scatter pads (0) map to sentinel group
            nc.vector.tensor_scalar(
                out=bneg[:, 0:S4], in0=vcomp[:, 0:S4], scalar1=0.5, scalar2=float(SENT) + 1.0,
                op0=Alu.is_lt, op1=Alu.mult,
            )
            nc.vector.scalar_tensor_tensor(
                out=bneg[:, 0:S4], in0=vcomp[:, 0:S4], scalar=1.0, in1=bneg[:, 0:S4],
                op0=Alu.mult, op1=Alu.add,
            )
            gix = v16[:, 0:S4]
            nc.vector.tensor_scalar(out=gix, in0=bneg[:, 0:S4], scalar1=-1.0, scalar2=None, op0=Alu.add)

            # ---- E: gather candidate payloads from xh (two halves) ----
            cand = t["cand"]
            for lo_s, hi_s in ((0, HF // 64), (HF // 64, S4)):
                nc.gpsimd.ap_gather(
                    cand[:, 64 * lo_s : 64 * hi_s].rearrange("p (a d) -> p a d", d=4),
                    xh[:].rearrange("p (a d) -> p a d", d=4),
                    gix[:, lo_s:hi_s],
                    channels=128,
                    num_elems=DW // 4,
                    d=4,
                    num_idxs=16 * (hi_s - lo_s),
                )

        def phase_F(b):
            t = tiles[b]
            cand, wk, sc = t["cand"], t["wk"], t["sc"]
            gA, gB = gtiles[b]
            th = sc[:, 1:2]
            nu = sc[:, 2:3]
            r1 = sc[:, 7:8]
            one_t = sc[:, 0:1]   # reuses m_s slot (dead after phase A)
            S1s = sc[:, 112:115]
            S2s = sc[:, 115:118]
            S3s = sc[:, 118:121]
            S1 = sc[:, 121:122]
            S2 = sc[:, 122:123]
            S3 = sc[:, 123:124]
            dlt = sc[:, 124:125]
            e1 = sc[:, 125:126]
            tb_ = sc[:, 126:127]
            lo = sc[:, 15:16]

            # Taylor-fused stats: ONE pass over cand computing S1,S2 (scalar
            # activation accumulators) and S3 (vector mult+reduce) at theta0 =
            # th (the newton lower bound incl. -EPS), then a Newton-corrected
            # closed form.  d loss/d th = 1 - S2/4 -> 0 at theta*, so the
            # single correction reaches ~1e-4 accuracy.
            for i in range(3):
                sl = slice(1280 * i, 1280 * (i + 1))
                nc.scalar.activation(
                    wk[:, 0:1280], cand[:, sl], Act.Relu, bias=nu, scale=1.0,
                    accum_out=S1s[:, i : i + 1],
                )
                nc.scalar.activation(cand[:, sl], wk[:, 0:1280], Act.Square,
                                     accum_out=S2s[:, i : i + 1])
                nc.vector.tensor_tensor(
                    out=cand[:, sl], in0=wk[:, 0:1280], in1=cand[:, sl], op=Alu.mult
                )
                nc.vector.tensor_reduce(S3s[:, i : i + 1], cand[:, sl], axis=Ax.X, op=Alu.add)

            nc.vector.tensor_reduce(S1, S1s, axis=Ax.X, op=Alu.add)
            nc.vector.tensor_reduce(S2, S2s, axis=Ax.X, op=Alu.add)
            nc.vector.tensor_reduce(S3, S3s, axis=Ax.X, op=Alu.add)

            nc.vector.reciprocal(r1, S1)
            nc.vector.tensor_scalar(
                out=dlt, in0=S2, scalar1=-4.0, scalar2=0.5, op0=Alu.add, op1=Alu.mult
            )
            nc.vector.tensor_tensor(out=dlt, in0=dlt, in1=r1, op=Alu.mult)

            # x[target] one-hot dot; one_t (==1.0) depends on the newton chain so
            # the static scheduler cannot hoist these to the head of the queue
            nc.vector.tensor_scalar(
                out=one_t, in0=r1, scalar1=0.0, scalar2=1.0, op0=Alu.mult, op1=Alu.add
            )
            nc.vector.scalar_tensor_tensor(
                out=gB[:], in0=gA[:], scalar=one_t, in1=gB[:], op0=Alu.mult, op1=Alu.mult
            )
            nc.vector.tensor_reduce(xtg[:, b : b + 1], gB[:], axis=Ax.X, op=Alu.add)

            # S3c = S3 - 3*dlt*(S2 - S1*dlt); loss = 4/3 + S3c/12 + th+dlt - x_t
            nc.vector.tensor_tensor(out=e1, in0=S1, in1=dlt, op=Alu.mult)
            nc.vector.tensor_tensor(out=e1, in0=S2, in1=e1, op=Alu.subtract)
            nc.vector.tensor_tensor(out=e1, in0=e1, in1=dlt, op=Alu.mult)
            nc.vector.scalar_tensor_tensor(
                out=S3, in0=e1, scalar=-3.0, in1=S3, op0=Alu.mult, op1=Alu.add
            )
            nc.vector.tensor_tensor(out=tb_, in0=th, in1=dlt, op=Alu.add)
            nc.vector.scalar_tensor_tensor(
                out=tb_, in0=S3, scalar=1.0 / 12.0, in1=tb_, op0=Alu.mult, op1=Alu.add
            )
            nc.vector.scalar_tensor_tensor(
                out=lo, in0=tb_, scalar=4.0 / 3.0, in1=xtg[:, b : b + 1],
                op0=Alu.add, op1=Alu.subtract,
            )
            nc.sync.dma_start(out[bass.ts(b, 128)], lo)

        phase_A(0)
        nc.sync.dma_start(iot[:], iotd)
        nc.sync.dma_start(wt[:], wseld)
        phase_BCDE(0)
        phase_A(1)
        phase_F(0)
        phase_BCDE(1)
        phase_F(1)

    nc.compile()
    return nc


def get_nc():
    if "nc" not in _nc_cache:
        _nc_cache["nc"] = _build_nc()
    return _nc_cache["nc"]


def make_in_maps(X, target):
    import ml_dtypes

    X = np.ascontiguousarray(np.asarray(X, dtype=np.float32))
    target = np.asarray(target).astype(np.int64)

    # wrapped gid+1 iota: iot[p, f] = 16*f + (p % 16) + 1
    pp, ff = np.meshgrid(np.arange(128), np.arange(WF), indexing="ij")
    iot = (16 * ff + (pp % 16) + 1).astype(np.float32)
    # residue-selection matrices: wsel[p, w, n] = 1 if n == 16*(p//16) + w
    wsel = np.zeros((128, 16, 128), np.float32)
    for w in range(16):
        for p in range(128):
            wsel[p, w, 16 * (p // 16) + w] = 1.0
    wsel = wsel.reshape(128, 16 * 128).astype(ml_dtypes.bfloat16)

    in_maps = []
    for k in range(N_CORES):
        Xk = X[k * ROWS : (k + 1) * ROWS]
        tk = target[k * ROWS : (k + 1) * ROWS]
        ohk = np.zeros((ROWS, 64), np.float32)
        ohk[np.arange(ROWS), (tk % 64).astype(np.int64)] = 1.0
        tblk = np.zeros((128, 16), np.int16)
        for c in range(4):
            rows = np.arange(64)
            vals = (rows * (V // 64) + (tk[64 * c + rows] // 64)).astype(np.int16)
            w = np.zeros((16, 4), np.int16)
            w[rows % 16, rows // 16] = vals
            tblk[:, 4 * c : 4 * (c + 1)] = np.tile(w, (8, 1))
        in_maps.append({"x": Xk, "oh": ohk, "tbl": tblk, "iot": iot, "wsel": wsel})
    return in_maps


def kernel(X, target):
    from concourse.bass_utils import run_bass_kernel_spmd

    nc = get_nc()
    in_maps = make_in_maps(X, target)
    res = run_bass_kernel_spmd(nc, in_maps, core_ids=list(range(N_CORES)))
    loss = np.concatenate([r["loss"] for r in res.results]).astype(np.float32)
    return loss



# revision 13
# speedup vs baseline: 1.3882x; 1.0485x over previous
"""Exact entmax-1.5 loss kernel for Trainium2 (8 NeuronCores, data-parallel over rows).

Algorithm (per row of X [N=2048, V=32000] f32):
  The entmax-1.5 threshold tau* solves  sum_j relu(X_j/2 - tau)^2 = 1.
  In X-units (theta = 2*tau):            sum_j relu(X_j - theta)^2 = 4.
  f(theta) is convex decreasing; Newton from a lower bound converges
  monotonically from below - no sort needed.

  v4 pipeline per 128-row block (fp16 payloads / bf16 bounds):
    A. Stream X in 16 f32 column chunks (DMA from SP queue); scalar converts
       to resident fp16 xh; vector builds group-of-4 maxes g4 bf16 and
       (per 4-chunk quarter) group-of-64 maxes g64 bf16.  A 4-iteration
       partial Newton on the first 3 quarters of g64 warm-starts theta while
       the last quarter still streams.
    B. 3 full all-vector Newton iterations on g64 -> theta_lb.
    C. mask = relu(g4 - theta_lb) in place; 16 accumulating 500-col bf16
       matmuls with residue-selection weights -> per-cluster group activity
       in wrapped [128,500] PSUM layout.
    D. Compaction: iota*mask, scan -> ranks, local_scatter -> 64 gids per
       partition, sentinel fixup (f16 mask/cum scratch).
    E. Two ap_gathers (512 idxs each) pull 4-fp16 payloads of the
       cluster-union candidate groups from xh -> cand [128,4096] fp16.
    F. Taylor-fused stats: ONE pass over cand computing S1,S2 (scalar
       activation accumulators) and S3 (vector mult+reduce) at theta0, then
       the Newton-corrected closed form delta=(S2-4)/(2*S1),
       S3c = S3 - 3*delta*(S2 - S1*delta),
       loss = 4/3 + S3c/12 + theta0 + delta - X[target]  (d loss/d theta =
       1 - S2/4 -> 0 at theta*, so one correction reaches ~1e-4; X[target]
       via up-front dma_gather + one-hot dot, f32 exact).

  Emission order software-pipelines the two blocks: A0, B0..E0, A1, F0,
  B1..E1, F1 so block 1 streams under block 0's tail and the two ~30us
  gpsimd gathers overlap other engines' work.

Host wrapper shards rows 256-per-core across 8 cores, no collectives.
"""
import numpy as np
from contextlib import ExitStack

N, V = 2048, 32000
N_CORES = 8
ROWS = N // N_CORES          # 256 rows per core
CHUNK = 1600
NCHUNK = V // CHUNK          # 20
NG4 = V // 4                 # 8000 groups of 4
WF = 500                     # wrapped cols: group = 16*F + (p % 16)
PART_ITERS = 4               # partial-newton iters on first 375 g64 cols
FULL_ITERS = 2
EPS_LB = 2e-2                # X-units safety margin (covers bf16 bound noise)
S4 = 60                      # per-partition capacity of compacted group ids
KU = 16 * S4                 # 1024 union groups per 16-partition cluster
CW = 4 * KU                  # 4096 compact width (fp16)
GPAD = 4
NGP = NG4 + GPAD             # 8008 groups incl. sentinel pad
DW = 4 * NGP                 # 32032 fp16 X width
SENT = NG4 + 2               # sentinel group id for scatter pads
HF = 2560                    # F-phase split: scalar [0:HF], vector [HF:CW]
NEG = -60000.0               # fp16-safe -inf substitute

_nc_cache = {}


def _build_nc():
    import concourse.bass as bass
    import concourse.bacc as bacc
    import concourse.tile as tile
    from concourse import mybir

    f32 = mybir.dt.float32
    f16 = mybir.dt.float16
    bf16 = mybir.dt.bfloat16
    i16 = mybir.dt.int16
    Alu = mybir.AluOpType
    Act = mybir.ActivationFunctionType
    Ax = mybir.AxisListType

    nc = bacc.Bacc("TRN2", target_bir_lowering=False, debug=False)
    x = nc.dram_tensor("x", [ROWS, V], f32, kind="ExternalInput").ap()
    oh = nc.dram_tensor("oh", [ROWS, 64], f32, kind="ExternalInput").ap()
    tbl = nc.dram_tensor("tbl", [128, 16], i16, kind="ExternalInput").ap()
    iotd = nc.dram_tensor("iot", [128, WF], f32, kind="ExternalInput").ap()
    wseld = nc.dram_tensor("wsel", [128, 16 * 128], bf16, kind="ExternalInput").ap()
    out = nc.dram_tensor("loss", [ROWS], f32, kind="ExternalOutput").ap()

    with tile.TileContext(nc) as tc, ExitStack() as ctx:
        const = ctx.enter_context(tc.tile_pool(name="const", bufs=1))
        big = ctx.enter_context(tc.tile_pool(name="big", bufs=1))
        psum = ctx.enter_context(tc.tile_pool(name="psum", bufs=2, space="PSUM"))

        iot = const.tile([128, WF], f32, tag="iot")
        wt = const.tile([128, 16 * 128], bf16, tag="wsel")
        xtg = const.tile([128, 2], f32, tag="xtg")
        tbl_t = const.tile([128, 16], i16, tag="tbl")

        nc.sync.dma_start(tbl_t[:], tbl[:, :])

        # ---- x[target] DMA gathers for both blocks, up front (gpsimd mlp lib).
        # The one-hot dot products are deferred to phase F to keep the vector
        # queue free for streaming.
        gtiles = []
        for b in range(2):
            gA = big.tile([128, 64], f32, tag="gA", bufs=2)
            gB = big.tile([128, 64], f32, tag="gB", bufs=2)
            for half, gdst in ((0, gA), (1, gB)):
                c = 2 * b + half
                src = x[64 * c : 64 * (c + 1), :].rearrange(
                    "r (bk e) -> (r bk) e", e=64
                )
                nc.gpsimd.dma_gather(
                    gdst.rearrange("p (one e) -> p one e", one=1),
                    src,
                    tbl_t[:, 4 * c : 4 * (c + 1)],
                    num_idxs=64,
                    num_idxs_reg=64,
                    elem_size=64,
                )
            nc.gpsimd.dma_start(gA[64:128, :], gB[0:64, :])
            # after the combine, gB is reloaded with the one-hot rows; issued
            # from the gpsimd queue so the dependency cannot block sync's
            # in-order chunk-trigger stream
            nc.gpsimd.dma_start(gB[:], oh[bass.ts(b, 128), :])
            gtiles.append((gA, gB))

        # dummy 16-idx ap_gather: pre-loads the Q7 ap_gather library while
        # block 0 is still streaming, so the real gathers start instantly
        sc0_w = big.tile([128, 80], f32, tag="warm", bufs=1)
        wix = sc0_w[:, 79:80].bitcast(i16)[:, 0:1]
        nc.vector.memset(wix, 0)
        # reads gB of block 1 (last mlp-gather output) so this runs after the
        # mlp library is done with it, keeping ap_gather resident for APG0
        nc.gpsimd.ap_gather(
            sc0_w[:, 0:64].rearrange("p (a d) -> p a d", d=4),
            gtiles[1][1][:, 0:16].rearrange("p (a d) -> p a d", d=4),
            wix,
            channels=128,
            num_elems=4,
            d=4,
            num_idxs=16,
        )

        # per-block tiles (tag rotation gives block-alternating buffers)
        tiles = []
        for b in range(2):
            t = {}
            t["xh"] = big.tile([128, DW], f16, name="xh", tag="xh", bufs=2)
            t["g4"] = big.tile([128, NG4], bf16, name="g4", tag="g4", bufs=2)
            t["cand"] = big.tile([128, CW], f16, name="cand", tag="cand", bufs=2)
            t["wk"] = big.tile([128, 1280], f16, name="wk", tag="wk", bufs=1)
            t["sc"] = big.tile([128, 128], f32, name="sc", tag="sc", bufs=2)
            t["scr"] = big.tile([128, 768], f32, name="scr", tag="scr", bufs=2)
            tiles.append(t)

        def g64_of(t):
            return t["scr"][:, 512:768].bitcast(bf16)

        def newton_iters(t, g, width, iters):
            sc, scr = t["sc"], t["scr"]
            th = sc[:, 1:2]
            nuB = sc[:, 2:3]
            S1a = sc[:, 3:4]
            S2a = sc[:, 4:5]
            r1 = sc[:, 7:8]
            dd = sc[:, 8:9]
            uB = scr[:, 0:256].bitcast(bf16)
            for _ in range(iters):
                nc.vector.tensor_scalar(out=nuB, in0=th, scalar1=-1.0, scalar2=None, op0=Alu.mult)
                nc.scalar.activation(
                    uB[:, 0:width], g[:, 0:width], Act.Relu, bias=nuB, scale=1.0,
                    accum_out=S1a,
                )
                nc.scalar.activation(uB[:, 0:width], uB[:, 0:width], Act.Square, accum_out=S2a)
                nc.vector.reciprocal(r1, S1a)
                nc.vector.tensor_scalar(
                    out=dd, in0=S2a, scalar1=-4.0, scalar2=0.5, op0=Alu.add, op1=Alu.mult
                )
                nc.vector.scalar_tensor_tensor(
                    out=th, in0=dd, scalar=r1, in1=th, op0=Alu.mult, op1=Alu.add
                )

        def phase_A(b):
            t = tiles[b]
            xh, g4, sc = t["xh"], t["g4"], t["sc"]
            g64 = g64_of(t)
            m_s = sc[:, 0:1]
            th = sc[:, 1:2]
            nc.vector.memset(xh[:, V:DW], NEG)
            xb = x[bass.ts(b, 128), :]
            for c in range(NCHUNK):
                xc = big.tile([128, CHUNK], f32, tag="xc", bufs=3)
                nc.sync.dma_start(xc[:], xb[:, bass.ts(c, CHUNK)])
                nc.scalar.activation(
                    xh[:, bass.ts(c, CHUNK)], xc[:], Act.Copy, bias=0.0, scale=1.0
                )
                nc.vector.tensor_reduce(
                    g4[:, bass.ts(c, CHUNK // 4)],
                    xh[:, bass.ts(c, CHUNK)].rearrange("p (g k) -> p g k", k=4),
                    axis=Ax.X,
                    op=Alu.max,
                )
                if c % 5 == 4:
                    q = c // 5
                    nc.vector.tensor_reduce(
                        g64[:, bass.ts(q, 125)],
                        g4[:, bass.ts(q, 2000)].rearrange("p (g k) -> p g k", k=16),
                        axis=Ax.X,
                        op=Alu.max,
                    )
                if c == 14:
                    # partial-newton warm start on quarters 0-2 (375 cols)
                    nc.vector.tensor_reduce(m_s, g64[:, 0:375], axis=Ax.X, op=Alu.max)
                    nc.vector.tensor_scalar(
                        out=th, in0=m_s, scalar1=-2.0, scalar2=None, op0=Alu.add
                    )
                    newton_iters(t, g64, 375, PART_ITERS)

        def phase_BCDE(b):
            t = tiles[b]
            xh, g4, sc, scr = t["xh"], t["g4"], t["sc"], t["scr"]
            g64 = g64_of(t)

            th = sc[:, 1:2]
            nu = sc[:, 2:3]
            vcomp = sc[:, 16:48].bitcast(i16)      # [128, 64] i16
            bneg = sc[:, 48:112]                    # [128, 64] f32

            maskv = scr[:, 0:250].bitcast(f16)      # [128, 500] f16
            cum = scr[:, 250:500].bitcast(f16)      # [128, 500] f16
            v16 = scr[:, 512:768].bitcast(i16)      # [128, 512] i16
            rank = scr[:, 0:256].bitcast(i16)       # [128, 512] i16, reuses maskv

            # ---- B: finish the G2 newton on the full 500 cols ----
            newton_iters(t, g64, WF, FULL_ITERS)
            nc.vector.tensor_scalar(out=th, in0=th, scalar1=-EPS_LB, scalar2=None, op0=Alu.add)
            nc.vector.tensor_scalar(out=nu, in0=th, scalar1=-1.0, scalar2=None, op0=Alu.mult)

            # ---- C: candidate mask (in place on g4) + residue matmuls ----
            nc.vector.tensor_scalar(
                out=g4[:], in0=g4[:], scalar1=th, scalar2=0.0,
                op0=Alu.subtract, op1=Alu.max,
            )
            pc = psum.tile([128, WF], f32, tag="pc")
            g4v = g4[:].rearrange("p (f wr) -> p f wr", wr=16)
            for w in range(16):
                nc.tensor.matmul(
                    pc[:],
                    wt[:, bass.ts(w, 128)],
                    g4v[:, :, w],
                    start=(w == 0),
                    stop=(w == 15),
                )

            # ---- D: compaction ----
            nc.vector.tensor_scalar(out=maskv, in0=pc[:], scalar1=0.0, scalar2=None, op0=Alu.is_gt)
            nc.vector.scalar_tensor_tensor(
                out=v16[:, 0:WF], in0=maskv, scalar=1.0, in1=iot[:],
                op0=Alu.mult, op1=Alu.mult,
            )
            nc.vector.tensor_tensor_scan(
                out=cum, data0=maskv, data1=maskv, initial=0.0,
                op0=Alu.add, op1=Alu.bypass,
            )
            nc.vector.tensor_tensor(out=cum, in0=cum, in1=maskv, op=Alu.mult)
            nc.vector.scalar_tensor_tensor(
                out=cum, in0=cum, scalar=float(S4) + 0.5, in1=cum,
                op0=Alu.is_le, op1=Alu.mult,
            )
            nc.vector.tensor_scalar(out=rank[:, 0:WF], in0=cum, scalar1=-1.0, scalar2=None, op0=Alu.add)
            nc.gpsimd.local_scatter(
                vcomp[:, 0:S4],
                v16[:, 0:WF],
                rank[:, 0:WF],
                channels=128,
                num_elems=S4,
                num_idxs=WF,
            )
            # group idx = (gid+1) - 1; scatter pads (0) map to sentinel group
            nc.vector.tensor_scalar(
                out=bneg[:, 0:S4], in0=vcomp[:, 0:S4], scalar1=0.5, scalar2=float(SENT) + 1.0,
                op0=Alu.is_lt, op1=Alu.mult,
            )
            nc.vector.scalar_tensor_tensor(
                out=bneg[:, 0:S4], in0=vcomp[:, 0:S4], scalar=1.0, in1=bneg[:, 0:S4],
                op0=Alu.mult, op1=Alu.add,
            )
            gix = v16[:, 0:S4]
            nc.vector.tensor_scalar(out=gix, in0=bneg[:, 0:S4], scalar1=-1.0, scalar2=None, op0=Alu.add)

            # ---- E: gather candidate payloads from xh (two halves) ----
            cand = t["cand"]
            for lo_s, hi_s in ((0, HF // 64), (HF // 64, S4)):
                nc.gpsimd.ap_gather(
                    cand[:, 64 * lo_s : 64 * hi_s].rearrange("p (a d) -> p a d", d=4),
                    xh[:].rearrange("p (a d) -> p a d", d=4),
                    gix[:, lo_s:hi_s],
                    channels=128,
                    num_elems=DW // 4,
                    d=4,
                    num_idxs=16 * (hi_s - lo_s),
                )

        def phase_F(b):
            t = tiles[b]
            cand, wk, sc = t["cand"], t["wk"], t["sc"]
            gA, gB = gtiles[b]
            th = sc[:, 1:2]
            nu = sc[:, 2:3]
            r1 = sc[:, 7:8]
            one_t = sc[:, 0:1]   # reuses m_s slot (dead after phase A)
            S1s = sc[:, 112:115]
            S2s = sc[:, 115:118]
            S3s = sc[:, 118:121]
            S1 = sc[:, 121:122]
            S2 = sc[:, 122:123]
            S3 = sc[:, 123:124]
            dlt = sc[:, 124:125]
            e1 = sc[:, 125:126]
            tb_ = sc[:, 126:127]
            lo = sc[:, 15:16]

            # Taylor-fused stats: ONE pass over cand computing S1,S2 (scalar
            # activation accumulators) and S3 (vector mult+reduce) at theta0 =
            # th (the newton lower bound incl. -EPS), then a Newton-corrected
            # closed form.  d loss/d th = 1 - S2/4 -> 0 at theta*, so the
            # single correction reaches ~1e-4 accuracy.
            for i in range(3):
                sl = slice(1280 * i, 1280 * (i + 1))
                nc.scalar.activation(
                    wk[:, 0:1280], cand[:, sl], Act.Relu, bias=nu, scale=1.0,
                    accum_out=S1s[:, i : i + 1],
                )
                nc.scalar.activation(cand[:, sl], wk[:, 0:1280], Act.Square,
                                     accum_out=S2s[:, i : i + 1])
                nc.vector.tensor_tensor(
                    out=cand[:, sl], in0=wk[:, 0:1280], in1=cand[:, sl], op=Alu.mult
                )
                nc.vector.tensor_reduce(S3s[:, i : i + 1], cand[:, sl], axis=Ax.X, op=Alu.add)

            nc.vector.tensor_reduce(S1, S1s, axis=Ax.X, op=Alu.add)
            nc.vector.tensor_reduce(S2, S2s, axis=Ax.X, op=Alu.add)
            nc.vector.tensor_reduce(S3, S3s, axis=Ax.X, op=Alu.add)

            nc.vector.reciprocal(r1, S1)
            nc.vector.tensor_scalar(
                out=dlt, in0=S2, scalar1=-4.0, scalar2=0.5, op0=Alu.add, op1=Alu.mult
            )
            nc.vector.tensor_tensor(out=dlt, in0=dlt, in1=r1, op=Alu.mult)

            # x[target] one-hot dot; one_t (==1.0) depends on the newton chain so
            # the static scheduler cannot hoist these to the head of the queue
            nc.vector.tensor_scalar(
                out=one_t, in0=r1, scalar1=0.0, scalar2=1.0, op0=Alu.mult, op1=Alu.add
            )
            nc.vector.scalar_tensor_tensor(
                out=gB[:], in0=gA[:], scalar=one_t, in1=gB[:], op0=Alu.mult, op1=Alu.mult
            )
            nc.vector.tensor_reduce(xtg[:, b : b + 1], gB[:], axis=Ax.X, op=Alu.add)

            # S3c = S3 - 3*dlt*(S2 - S1*dlt); loss = 4/3 + S3c/12 + th+dlt - x_t
            nc.vector.tensor_tensor(out=e1, in0=S1, in1=dlt, op=Alu.mult)
            nc.vector.tensor_tensor(out=e1, in0=S2, in1=e1, op=Alu.subtract)
            nc.vector.tensor_tensor(out=e1, in0=e1, in1=dlt, op=Alu.mult)
            nc.vector.scalar_tensor_tensor(
                out=S3, in0=e1, scalar=-3.0, in1=S3, op0=Alu.mult, op1=Alu.add
            )
            nc.vector.tensor_tensor(out=tb_, in0=th, in1=dlt, op=Alu.add)
            nc.vector.scalar_tensor_tensor(
                out=tb_, in0=S3, scalar=1.0 / 12.0, in1=tb_, op0=Alu.mult, op1=Alu.add
            )
            nc.vector.scalar_tensor_tensor(
                out=lo, in0=tb_, scalar=4.0 / 3.0, in1=xtg[:, b : b + 1],
                op0=Alu.add, op1=Alu.subtract,
            )
            nc.sync.dma_start(out[bass.ts(b, 128)], lo)

        phase_A(0)
        nc.sync.dma_start(iot[:], iotd)
        nc.sync.dma_start(wt[:], wseld)
        phase_BCDE(0)
        phase_A(1)
        phase_F(0)
        phase_BCDE(1)
        phase_F(1)

    nc.compile()
    return nc


def get_nc():
    if "nc" not in _nc_cache:
        _nc_cache["nc"] = _build_nc()
    return _nc_cache["nc"]


def make_in_maps(X, target):
    import ml_dtypes

    X = np.ascontiguousarray(np.asarray(X, dtype=np.float32))
    target = np.asarray(target).astype(np.int64)

    # wrapped gid+1 iota: iot[p, f] = 16*f + (p % 16) + 1
    pp, ff = np.meshgrid(np.arange(128), np.arange(WF), indexing="ij")
    iot = (16 * ff + (pp % 16) + 1).astype(np.float32)
    # residue-selection matrices: wsel[p, w, n] = 1 if n == 16*(p//16) + w
    wsel = np.zeros((128, 16, 128), np.float32)
    for w in range(16):
        for p in range(128):
            wsel[p, w, 16 * (p // 16) + w] = 1.0
    wsel = wsel.reshape(128, 16 * 128).astype(ml_dtypes.bfloat16)

    in_maps = []
    for k in range(N_CORES):
        Xk = X[k * ROWS : (k + 1) * ROWS]
        tk = target[k * ROWS : (k + 1) * ROWS]
        ohk = np.zeros((ROWS, 64), np.float32)
        ohk[np.arange(ROWS), (tk % 64).astype(np.int64)] = 1.0
        tblk = np.zeros((128, 16), np.int16)
        for c in range(4):
            rows = np.arange(64)
            vals = (rows * (V // 64) + (tk[64 * c + rows] // 64)).astype(np.int16)
            w = np.zeros((16, 4), np.int16)
            w[rows % 16, rows // 16] = vals
            tblk[:, 4 * c : 4 * (c + 1)] = np.tile(w, (8, 1))
        in_maps.append({"x": Xk, "oh": ohk, "tbl": tblk, "iot": iot, "wsel": wsel})
    return in_maps


def kernel(X, target):
    from concourse.bass_utils import run_bass_kernel_spmd

    nc = get_nc()
    in_maps = make_in_maps(X, target)
    res = run_bass_kernel_spmd(nc, in_maps, core_ids=list(range(N_CORES)))
    loss = np.concatenate([r["loss"] for r in res.results]).astype(np.float32)
    return loss

